# revision 45
# baseline (speedup 1.0000x reference)
"""Trainium2 Bass kernel for the DeltaNet-style block nn_Block_68341519614809.

All-bf16 matmuls (1 cyc/row vs fp32-HIGH ~2.4), SBUF-resident q/k/v (no
DRAM staging), A^4-truncated triangular inverse (validated vs fp64 numpy:
8.3e-3 rel, tol 2e-2), scale folding (raw q with r_q folded into the RMS
scale, g_rms into Wo, -beta*r_k into KbTn), reciprocal_approx_fast for
rsqrt chains, x-residual via identity-matmul into the output-projection
psum (0.5x on each pair core), bf16 ReduceScatter in blocks of [4,4,4,2,2]
chunks with LayerNorm deferred 2 chunks (gpsimd-queue DMAs), bf16 y_out
(host casts to fp32).

Sharding: core c = 2*b + g  (b in 0..3 batch, g in 0..1 head-group of 2
heads).  ReduceScatter over core pairs; each core LayerNorms its
half-rows; host gathers.
"""
import numpy as np

B, T_FULL, D, H, CONV_K = 4, 2048, 1024, 4, 4
DH = 256          # head dim
DG = 512          # head-group width (2 heads)
EPS = 1e-5
CK = 128          # delta-rule chunk size
TB = 256          # stage-A t-block
N_CORES = 8

_cache = {}


def _rs_blocks(T):
    """(start_chunk, end_chunk) per ReduceScatter block; finer at the tail."""
    n_ck = T // CK
    if n_ck == 16:
        return [(0, 4), (4, 8), (8, 12), (12, 14), (14, 16)]
    return [(i, i + n_ck // 2) for i in range(0, n_ck, n_ck // 2)]


def _rows_idx(T):
    """Global t-rows owned by head-group g under the blocked ReduceScatter."""
    import numpy as _np
    segs = []
    for s, e in _rs_blocks(T):
        half = (e - s) * CK // 2
        segs.append((s * CK, half))
    out = {}
    for g in range(2):
        out[g] = _np.concatenate(
            [_np.arange(t0 + g * half, t0 + (g + 1) * half)
             for t0, half in segs])
    return out


def _build(T=T_FULL):
    from contextlib import ExitStack
    import concourse.bacc as bacc
    import concourse.tile as tile
    import concourse.mybir as mybir

    F32 = mybir.dt.float32
    BF = mybir.dt.bfloat16
    ALU = mybir.AluOpType
    ACTF = mybir.ActivationFunctionType

    n_tb = T // TB
    n_ck = T // CK
    TH = T // 2
    seg = TB + 4

    nc = bacc.Bacc("TRN2", target_bir_lowering=False, debug=False,
                   num_devices=N_CORES)

    # ---- I/O (all bf16 unless noted) ----
    xt = nc.dram_tensor("xt", [D, T + 4], BF, kind="ExternalInput")
    xtm = nc.dram_tensor("xtm", [T, D], BF, kind="ExternalInput")  # 0.5*x[b]
    wq = nc.dram_tensor("wq", [D, DG], BF, kind="ExternalInput")
    wk = nc.dram_tensor("wk", [D, DG], BF, kind="ExternalInput")
    wv = nc.dram_tensor("wv", [D, DG], BF, kind="ExternalInput")
    wb2 = nc.dram_tensor("wb2", [D, 2], BF, kind="ExternalInput")
    cqT = nc.dram_tensor("cqT", [DG, CONV_K], F32, kind="ExternalInput")
    ckT = nc.dram_tensor("ckT", [DG, CONV_K], F32, kind="ExternalInput")
    cvT = nc.dram_tensor("cvT", [DG, CONV_K], F32, kind="ExternalInput")
    # diag(c0..c3) per j-tile, for the PE-side conv taps
    cdq = nc.dram_tensor("cdq", [128, 16 * 128], BF, kind="ExternalInput")
    cdk = nc.dram_tensor("cdk", [128, 16 * 128], BF, kind="ExternalInput")
    cdv = nc.dram_tensor("cdv", [128, 16 * 128], BF, kind="ExternalInput")
    wo = nc.dram_tensor("wo", [DG, D], BF, kind="ExternalInput")  # g_rms folded
    lng = nc.dram_tensor("lng", [128, D], BF, kind="ExternalInput")
    lnb = nc.dram_tensor("lnb", [128, D], BF, kind="ExternalInput")
    ident_in = nc.dram_tensor("ident", [128, 128], BF, kind="ExternalInput")
    id2_in = nc.dram_tensor("id2f", [2, 2], F32, kind="ExternalInput")
    ones_in = nc.dram_tensor("ones", [128, 128], BF, kind="ExternalInput")
    mlo_in = nc.dram_tensor("mlo", [128, 128], BF, kind="ExternalInput")  # +(j<i)
    mup_in = nc.dram_tensor("mup", [128, 128], BF, kind="ExternalInput")  # +(j>i)
    mui_in = nc.dram_tensor("mui", [128, 128], BF, kind="ExternalInput")  # (j>=i)
    y_out = nc.dram_tensor("y_out", [TH, D], BF, kind="ExternalOutput")

    with tile.TileContext(nc) as tc, ExitStack() as top:
        top.enter_context(nc.allow_low_precision(
            reason="bf16 pipeline validated 8.3e-3 rel vs 2e-2 tol in sim"))
        const = top.enter_context(tc.tile_pool(name="const", bufs=1))
        psum = top.enter_context(tc.tile_pool(name="psum", bufs=2, space="PSUM"))
        dram = top.enter_context(tc.tile_pool(name="dram", bufs=1, space="DRAM"))

        def ps_pay():
            return psum.tile([128, 512], F32, tag="pay", name="pay")

        def ps_pg():
            return psum.tile([128, 256], F32, tag="pg", name="pg")

        def ps_med():
            return psum.tile([128, 512], F32, tag="pmed", name="pmed")

        def ps_small():
            return psum.tile([128, 512], F32, tag="psmall", name="psmall")

        # ---- stage-A weights + x first (startup critical path) ----
        wpool = top.enter_context(tc.tile_pool(name="wpool", bufs=1))
        xbp = top.enter_context(tc.tile_pool(name="xbp", bufs=2))
        WT = {}

        def load_w(nm, wsrc):
            wt = wpool.tile([128, 8 * DG], BF, tag=f"w{nm}", name=f"w{nm}")
            nc.sync.dma_start(wt[:].rearrange("p (k d) -> p k d", k=8),
                              wsrc[:].rearrange("(k p) d -> p k d", p=128))
            WT[nm] = wt

        def load_xb(t0):
            xb = xbp.tile([128, 8 * seg], BF, tag="xb", name="xb")
            nc.sync.dma_start(
                xb[:].rearrange("p (k t) -> p k t", k=8),
                xt[:].rearrange("(k p) t -> p k t", p=128)[:, :, t0:t0 + seg])
            return xb

        load_w("q", wq)
        xb0 = load_xb(0)
        WB2 = wpool.tile([128, 16], BF, tag="wb2", name="wb2")
        nc.sync.dma_start(WB2[:].rearrange("p (k j) -> p k j", k=8),
                          wb2[:].rearrange("(k p) j -> p k j", p=128))
        load_w("k", wk)
        load_w("v", wv)

        # ---- constants ----
        IB = const.tile([128, 128], BF, tag="ib", name="ib")
        nc.sync.dma_start(IB[:], ident_in[:])
        ONES = const.tile([128, 128], BF, tag="ones", name="ones")
        nc.sync.dma_start(ONES[:], ones_in[:])
        MLOP2 = const.tile([128, 256], BF, tag="mlop", name="mlop")
        nc.sync.dma_start(MLOP2[:, 0:128], mlo_in[:])
        nc.sync.dma_start(MLOP2[:, 128:256], mlo_in[:])
        MUPP2 = const.tile([128, 256], BF, tag="mupp", name="mupp")
        nc.sync.dma_start(MUPP2[:, 0:128], mup_in[:])
        nc.sync.dma_start(MUPP2[:, 128:256], mup_in[:])
        MUI2 = const.tile([128, 256], BF, tag="mui", name="mui")
        nc.sync.dma_start(MUI2[:, 0:128], mui_in[:])
        nc.sync.dma_start(MUI2[:, 128:256], mui_in[:])
        EPS1 = const.tile([128, 1], F32, tag="eps1", name="eps1")
        nc.gpsimd.memset(EPS1[:], 1e-6)
        EPSL = const.tile([128, 1], F32, tag="epsl", name="epsl")
        nc.gpsimd.memset(EPSL[:], EPS)
        ONE32 = const.tile([1, 1], F32, tag="one32", name="one32")
        nc.gpsimd.memset(ONE32[:], 1.0)
        ID2 = const.tile([2, 2], F32, tag="id2", name="id2")
        nc.sync.dma_start(ID2[:], id2_in[:])
        CD = {}
        for nm, cd in (("q", cdq), ("k", cdk), ("v", cdv)):
            CD[nm] = const.tile([128, 16 * 128], BF, tag=f"cd{nm}", name=f"cd{nm}")
            nc.sync.dma_start(CD[nm][:], cd[:])

        # beta rows + q/k/v stay in SBUF (bf16, d-major, col = j*T + t)
        BT = [const.tile([1, T], F32, tag=f"BT{h}", name=f"BT{h}") for h in range(2)]
        QKV = {}
        for nm in ("q", "k", "v"):
            QKV[nm] = const.tile([128, 4 * T], BF, tag=f"qkv{nm}", name=f"qkv{nm}")

        # ================= stage A: projections + conv + silu =================
        cvp = top.enter_context(tc.tile_pool(name="cvp", bufs=3))

        def conv_tail(st):
            """all 4 conv taps as diag-matmuls on PE; silu straight from PSUM."""
            nm, j, t0, u = st
            ps2 = ps_pg()
            for tap in range(4):
                nc.tensor.matmul(ps2[:, 0:TB],
                                 CD[nm][:, (4 * j + tap) * 128:
                                           (4 * j + tap + 1) * 128],
                                 u[:, 1 + tap:TB + 1 + tap],
                                 start=(tap == 0), stop=(tap == 3))
            nc.scalar.activation(QKV[nm][:, j * T + t0:j * T + t0 + TB],
                                 ps2[:, 0:TB], ACTF.Silu)

        pend_conv = []

        def emit_tb(tb):
            t0 = tb * TB
            xb = xb0 if tb == 0 else load_xb(t0)
            for h in range(2):
                psb = ps_small()
                for k in range(8):
                    nc.tensor.matmul(psb[0:1, 0:TB],
                                     WB2[:, k * 2 + h:k * 2 + h + 1],
                                     xb[:, k * seg + 4:(k + 1) * seg],
                                     start=(k == 0), stop=(k == 7))
                bth = cvp.tile([1, TB], F32, tag="bth", name="bth")
                nc.scalar.activation(bth[0:1, 0:TB], psb[0:1, 0:TB],
                                     ACTF.Tanh, scale=0.5)
                nc.vector.tensor_scalar(BT[h][0:1, t0:t0 + TB], bth[0:1, 0:TB],
                                        0.5, 0.5, ALU.mult, ALU.add)

            for nm in ("q", "k", "v"):
                for j in range(4):
                    ps = ps_pay()
                    for k in range(8):
                        nc.tensor.matmul(ps[:, 0:seg],
                                         WT[nm][:, k * DG + j * 128:
                                                    k * DG + (j + 1) * 128],
                                         xb[:, k * seg:(k + 1) * seg],
                                         start=(k == 0), stop=(k == 7))
                    # conv[t] = sum_i cw[i]*pre[t-3+i]; ps col (t-t0+4)
                    u = cvp.tile([128, seg], BF, tag="u", name="u")
                    nc.vector.tensor_copy(u[:, 1:seg], ps[:, 1:seg])
                    if pend_conv:
                        conv_tail(pend_conv.pop())
                    pend_conv.append((nm, j, t0, u))

        for _tb in range(n_tb):
            emit_tb(_tb)
        while pend_conv:
            conv_tail(pend_conv.pop())

        # ================= chunk stage: delta rule =================
        ckx = top.enter_context(ExitStack())
        work = ckx.enter_context(tc.tile_pool(name="work", bufs=3))
        spool = ckx.enter_context(tc.tile_pool(name="spool", bufs=2))
        ohp = ckx.enter_context(tc.tile_pool(name="ohp", bufs=2))
        wop = ckx.enter_context(tc.tile_pool(name="wop", bufs=1))
        xcp = ckx.enter_context(tc.tile_pool(name="xcp", bufs=2))

        WO = [wop.tile([128, D], BF, tag=f"wo{k}", name=f"wo{k}") for k in range(4)]
        for k in range(4):
            nc.sync.dma_start(WO[k][:], wo[k * 128:(k + 1) * 128, :])

        ydr = dram.tile([T, D], BF, tag="ydr", name="ydr")

        S = {}
        for h in range(2):
            S[h] = spool.tile([128, 2 * DH], BF, tag=f"S{h}", name=f"S{h}")
            nc.gpsimd.memset(S[h][:], 0.0)

        blocks = _rs_blocks(T)
        ybase = []
        off = 0
        for s, e in blocks:
            half = (e - s) * CK // 2
            ybase.append((off, half))
            off += half
        yhb = [dram.tile([ybase[rb][1], D], BF, tag=f"yhb{rb}", name=f"yhb{rb}")
               for rb in range(len(blocks))]

        LNG = const.tile([128, D], BF, tag="lng", name="lng")
        nc.sync.dma_start(LNG[:], lng[:])
        LNB = const.tile([128, D], BF, tag="lnb", name="lnb")
        nc.sync.dma_start(LNB[:], lnb[:])
        lnp = ckx.enter_context(tc.tile_pool(name="lnp", bufs=2))

        def dt_ap(nm, h, i, cc):
            """single d-tile [128, 128]"""
            j = 2 * h + i
            return QKV[nm][:, j * T + cc.start:j * T + cc.stop]

        def prep2(c):
            """Per-chunk prep for BOTH heads; raw-k Gram with scales folded
            into per-partition columns (CB2) and row-broadcast masks."""
            cc = slice(c * CK, (c + 1) * CK)

            # -- l2 norm sums: SQ2 [q0|k0|q1|k1|q2|k2|q3|k3] (j = 2h+i) --
            SQ2 = work.tile([128, 1024], BF, tag="SQ", name="SQ", bufs=2)
            sqw = SQ2[:].rearrange("p (j t) -> p j t", j=4)
            qa = QKV["q"][:].rearrange("p (j t) -> p j t", j=4)[:, :, cc]
            ka = QKV["k"][:].rearrange("p (j t) -> p j t", j=4)[:, :, cc]
            nc.gpsimd.tensor_tensor(sqw[:, :, 0:128], qa, qa, ALU.mult)
            nc.gpsimd.tensor_tensor(sqw[:, :, 128:256], ka, ka, ALU.mult)
            psn = ps_small()  # [1,512] = [nq0|nk0|nq1|nk1]
            for h in range(2):
                for i in range(2):
                    nc.tensor.matmul(psn[0:1, h * 256:(h + 1) * 256],
                                     ONES[:, 0:1],
                                     SQ2[:, (2 * h + i) * 256:
                                            (2 * h + i + 1) * 256],
                                     start=(i == 0), stop=(i == 1))
            sqr = work.tile([1, 512], F32, tag="sqr", name="sqr")
            nc.scalar.activation(sqr[:], psn[0:1, 0:512], ACTF.Sqrt,
                                 bias=EPS1[0:1, 0:1])
            R3f = work.tile([1, 512], F32, tag="R3f", name="R3f")
            nc.vector.reciprocal_approx_fast(R3f[:], sqr[:])
            # RCAT [1,512] = [rk0|mb0|rk1|mb1]  (mb = -beta*rk)
            RCAT = work.tile([1, 512], BF, tag="RCAT", name="RCAT")
            rcw = RCAT[0:1].rearrange("p (i t) -> p i t", i=2)
            r3w = R3f[0:1].rearrange("p (i t) -> p i t", i=2)
            nc.gpsimd.tensor_copy(rcw[:, :, 0:128], r3w[:, :, 128:256])
            for h in range(2):
                nc.vector.scalar_tensor_tensor(
                    RCAT[0:1, h * 256 + 128:(h + 1) * 256],
                    R3f[0:1, h * 256 + 128:(h + 1) * 256],
                    -1.0, BT[h][0:1, cc], ALU.mult, ALU.mult)
            psbr = ps_small()
            nc.tensor.matmul(psbr[0:128, 0:512], ONES[0:1, :], RCAT[:],
                             start=True, stop=True)
            RB = work.tile([128, 512], BF, tag="RB", name="RB", bufs=2)
            nc.scalar.activation(RB[:], psbr[0:128, 0:512], ACTF.Copy)
            # columns: CB2 [128,6]: per h: rq(3h) b(3h+1) rk(3h+2); MBC: mb
            pst1 = ps_small()
            for h in range(2):
                nc.tensor.transpose(pst1[0:128, 3 * h:3 * h + 1],
                                    R3f[0:1, h * 256:h * 256 + 128],
                                    ONE32[0:1, 0:1])
                nc.tensor.transpose(pst1[0:128, 3 * h + 1:3 * h + 2],
                                    BT[h][0:1, cc], ONE32[0:1, 0:1])
                nc.tensor.transpose(pst1[0:128, 3 * h + 2:3 * h + 3],
                                    R3f[0:1, h * 256 + 128:(h + 1) * 256],
                                    ONE32[0:1, 0:1])
            pstb = pst1[:, 6:8].bitcast(BF)  # [128, 4] bf16 view
            for h in range(2):
                nc.tensor.transpose(pstb[0:128, 2 * h:2 * h + 1],
                                    RCAT[0:1, h * 256 + 128:(h + 1) * 256],
                                    ONES[0:1, 0:1])
            CB2 = work.tile([128, 6], F32, tag="CB", name="CB", bufs=6)
            nc.scalar.copy(CB2[:], pst1[0:128, 0:6])
            MBC = work.tile([128, 2], F32, tag="MBC", name="MBC", bufs=6)
            nc.vector.tensor_copy(
                MBC[:].rearrange("p (h x) -> p h x", x=1),
                pstb[0:128].rearrange("p (h x) -> p h x", h=2)[:, :, 0:1])
            cbh = CB2[:].rearrange("p (h x) -> p h x", h=2)
            r2dh2 = work.tile([128, 2], F32, tag="r2dh", name="r2dh", bufs=6)
            nc.vector.scalar_tensor_tensor(
                r2dh2[:].rearrange("p (h x) -> p h x", x=1),
                cbh[:, :, 0:1], 1.0 / DH, cbh[:, :, 0:1], ALU.mult, ALU.mult)

            # -- row-broadcast masks: MLORK = mlo*rk_rows, MUPBK = mup*mb_rows
            rb2 = RB[:].rearrange("p (i t) -> p i t", i=2)
            MLORK = work.tile([128, 256], BF, tag="MLORK", name="MLORK", bufs=2)
            nc.gpsimd.tensor_tensor(
                MLORK[:].rearrange("p (i t) -> p i t", i=2),
                MLOP2[:].rearrange("p (i t) -> p i t", i=2),
                rb2[:, :, 0:128], ALU.mult)
            MUPBK = work.tile([128, 256], BF, tag="MUPBK", name="MUPBK", bufs=2)
            nc.gpsimd.tensor_tensor(
                MUPBK[:].rearrange("p (i t) -> p i t", i=2),
                MUPP2[:].rearrange("p (i t) -> p i t", i=2),
                rb2[:, :, 128:256], ALU.mult)

            # -- Gram (symmetric, raw k): psg [G_h0 | G_h1] --
            psg = ps_pg()
            for h in range(2):
                for i in range(2):
                    nc.tensor.matmul(psg[:, h * 128:(h + 1) * 128],
                                     dt_ap("k", h, i, cc),
                                     dt_ap("k", h, i, cc),
                                     start=(i == 0), stop=(i == 1))
            An2 = work.tile([128, 256], BF, tag="An", name="An", bufs=2)
            ATn2 = work.tile([128, 256], BF, tag="ATn", name="ATn", bufs=2)
            for h in range(2):
                sl = slice(h * 128, (h + 1) * 128)
                nc.vector.scalar_tensor_tensor(An2[:, sl], psg[:, sl],
                                               MBC[:, h:h + 1],
                                               MLORK[:, sl], ALU.mult, ALU.mult)
                nc.vector.scalar_tensor_tensor(ATn2[:, sl], psg[:, sl],
                                               CB2[:, 3 * h + 2:3 * h + 3],
                                               MUPBK[:, sl], ALU.mult, ALU.mult)

            # -- truncated inverse transpose: TpT = (I+A4T)(I+A2T)(I-AT) --
            pp = ps_pay()  # [P2_0|P2T_0|P2_1|P2T_1]
            for h in range(2):
                sl = slice(h * 128, (h + 1) * 128)
                nc.tensor.matmul(pp[:, h * 256:h * 256 + 128], ATn2[:, sl],
                                 An2[:, sl], start=True, stop=True)
                nc.tensor.matmul(pp[:, h * 256 + 128:(h + 1) * 256],
                                 An2[:, sl], ATn2[:, sl], start=True, stop=True)
            PP = work.tile([128, 512], BF, tag="PP", name="PP", bufs=2)
            nc.scalar.copy(PP[:], pp[:, 0:512])
            pr1 = ps_pg()
            for h in range(2):
                sl = slice(h * 128, (h + 1) * 128)
                nc.tensor.matmul(pr1[:, sl], PP[:, h * 256:h * 256 + 128],
                                 PP[:, h * 256 + 128:(h + 1) * 256],
                                 start=True, stop=False)
                nc.tensor.matmul(pr1[:, sl], IB[:], IB[:], start=False,
                                 stop=True)
            R1 = work.tile([128, 256], BF, tag="R1", name="R1", bufs=2)
            nc.scalar.copy(R1[:], pr1[:, 0:256])
            pr2 = ps_pg()
            for h in range(2):
                sl = slice(h * 128, (h + 1) * 128)
                nc.tensor.matmul(pr2[:, sl], PP[:, h * 256:h * 256 + 128],
                                 R1[:, sl], start=True, stop=False)
                nc.tensor.matmul(pr2[:, sl], IB[:], R1[:, sl], start=False,
                                 stop=True)
            R2 = work.tile([128, 256], BF, tag="R2", name="R2", bufs=2)
            nc.scalar.copy(R2[:], pr2[:, 0:256])
            pr3 = ps_pg()
            for h in range(2):
                sl = slice(h * 128, (h + 1) * 128)
                nc.tensor.matmul(pr3[:, sl], An2[:, sl], R2[:, sl],
                                 start=True, stop=False)
                nc.tensor.matmul(pr3[:, sl], IB[:], R2[:, sl], start=False,
                                 stop=True)
            TTt2 = work.tile([128, 256], BF, tag="TTt", name="TTt", bufs=5)
            nc.scalar.copy(TTt2[:], pr3[:, 0:256])

            # -- MT' = triu(k_raw^T q_raw) (rk rides on U') --
            psmt = ps_pg()
            for h in range(2):
                for i in range(2):
                    nc.tensor.matmul(psmt[:, h * 128:(h + 1) * 128],
                                     dt_ap("k", h, i, cc),
                                     dt_ap("q", h, i, cc),
                                     start=(i == 0), stop=(i == 1))
            MT2 = work.tile([128, 256], BF, tag="MT", name="MT", bufs=5)
            nc.vector.tensor_tensor(MT2[:], psmt[:, 0:256], MUI2[:], ALU.mult)

            # -- beta*V (t-major) via PE transpose --
            vw = ps_med()
            vwb = vw[:, 0:256].bitcast(BF)  # [128, 512] bf16 view
            for h in range(2):
                for i in range(2):
                    nc.tensor.transpose(vwb[:, (2 * h + i) * 128:
                                               (2 * h + i + 1) * 128],
                                        dt_ap("v", h, i, cc), IB[:])
            Vtb2 = work.tile([128, 512], BF, tag="Vtb", name="Vtb", bufs=5)
            for h in range(2):
                nc.scalar.activation(Vtb2[:, h * 256:(h + 1) * 256],
                                     vwb[:, h * 256:(h + 1) * 256],
                                     ACTF.Copy, scale=CB2[:, 3 * h + 1:3 * h + 2])

            # -- raw K t-major (rk rides on U') --
            pskt = ps_med()
            psktb = pskt[:, 0:256].bitcast(BF)
            for h in range(2):
                for i in range(2):
                    nc.tensor.transpose(psktb[:, (2 * h + i) * 128:
                                                (2 * h + i + 1) * 128],
                                        dt_ap("k", h, i, cc), IB[:])
            Kh2 = work.tile([128, 512], BF, tag="Kh", name="Kh", bufs=5)
            nc.vector.tensor_copy(Kh2[:], psktb[:])

            return dict(TTt=TTt2, Vtb=Vtb2, MT=MT2, Kh=Kh2, CB=CB2,
                        MBC=MBC, r2dh=r2dh2)

        def spart2(c, Pd):
            cc = slice(c * CK, (c + 1) * CK)
            TTt2, Vtb2, MT2 = Pd["TTt"], Pd["Vtb"], Pd["MT"]
            Kh2, CB2, r2dh2 = Pd["Kh"], Pd["CB"], Pd["r2dh"]
            MBC = Pd["MBC"]

            # kS = k_raw^T S; W2b = (kS * mb) + beta*Vt  (mb = -beta*rk)
            psw = ps_med()
            for h in range(2):
                for i in range(2):
                    nc.tensor.matmul(psw[:, h * 256:(h + 1) * 256],
                                     dt_ap("k", h, i, cc),
                                     S[h][:, i * 256:(i + 1) * 256],
                                     start=(i == 0), stop=(i == 1))
            W2b2 = work.tile([128, 512], BF, tag="W2b", name="W2b", bufs=2)
            for h in range(2):
                sl = slice(h * 256, (h + 1) * 256)
                nc.vector.scalar_tensor_tensor(W2b2[:, sl], psw[:, sl],
                                               MBC[:, h:h + 1],
                                               Vtb2[:, sl], ALU.mult, ALU.add)
            # U' = rk * (Tp' W2b)
            pu = ps_med()
            for h in range(2):
                sl = slice(h * 256, (h + 1) * 256)
                nc.tensor.matmul(pu[:, sl], TTt2[:, h * 128:(h + 1) * 128],
                                 W2b2[:, sl], start=True, stop=True)
            U2 = work.tile([128, 512], BF, tag="U", name="U", bufs=2)
            for h in range(2):
                sl = slice(h * 256, (h + 1) * 256)
                nc.scalar.activation(U2[:, sl], pu[:, sl], ACTF.Copy,
                                     scale=CB2[:, 3 * h + 2:3 * h + 3])

            # O_raw (t-major) = q_raw S + MT' U'; per-head RMS, r_q folded
            pso = ps_med()
            for h in range(2):
                sl = slice(h * 256, (h + 1) * 256)
                for i in range(2):
                    nc.tensor.matmul(pso[:, sl], dt_ap("q", h, i, cc),
                                     S[h][:, i * 256:(i + 1) * 256],
                                     start=(i == 0), stop=False)
                nc.tensor.matmul(pso[:, sl], MT2[:, h * 128:(h + 1) * 128],
                                 U2[:, sl], start=False, stop=True)
            waste = work.tile([128, 512], BF, tag="waste", name="waste", bufs=1)
            sso2 = work.tile([128, 2], F32, tag="sso", name="sso")
            for h in range(2):
                nc.scalar.activation(waste[:, h * 256:(h + 1) * 256],
                                     pso[:, h * 256:(h + 1) * 256],
                                     ACTF.Square, accum_out=sso2[:, h:h + 1])
            ssp = work.tile([128, 2], F32, tag="ssp", name="ssp")
            nc.vector.tensor_tensor(ssp[:], sso2[:], r2dh2[:], ALU.mult)
            sdo = work.tile([128, 2], F32, tag="sdo", name="sdo")
            nc.scalar.activation(sdo[:], ssp[:], ACTF.Sqrt, bias=EPSL[:])
            rcoi = work.tile([128, 2], F32, tag="rcoi", name="rcoi")
            nc.vector.reciprocal_approx_fast(rcoi[:], sdo[:])
            rco = work.tile([128, 2], F32, tag="rco", name="rco")
            cbh = CB2[:].rearrange("p (h x) -> p h x", h=2)
            nc.vector.tensor_tensor(
                rco[:].rearrange("p (h x) -> p h x", x=1),
                rcoi[:].rearrange("p (h x) -> p h x", x=1),
                cbh[:, :, 0:1], ALU.mult)
            Ohn2 = work.tile([128, 512], BF, tag="Ohn", name="Ohn", bufs=2)
            for h in range(2):
                sl = slice(h * 256, (h + 1) * 256)
                nc.scalar.activation(Ohn2[:, sl], pso[:, sl], ACTF.Copy,
                                     scale=rco[:, h:h + 1])
            psot = ps_pg()
            psob = psot[:, 0:256].bitcast(BF)  # [128, 512] bf16 view
            for jj in range(4):
                nc.tensor.transpose(psob[:, jj * 128:(jj + 1) * 128],
                                    Ohn2[:, jj * 128:(jj + 1) * 128], IB[:])
            OhT2 = ohp.tile([128, 512], BF, tag="OhT", name="OhT")
            nc.scalar.copy(OhT2[:], psob[:])

            # S += k_raw^T U'  (skip on the final chunk)
            if c + 1 < n_ck:
                for h in range(2):
                    ktds = ps_med() if h == 0 else ps_pay()
                    for i in range(2):
                        reg = slice(i * 256, (i + 1) * 256)
                        nc.tensor.matmul(ktds[:, reg],
                                         Kh2[:, (2 * h + i) * 128:
                                                (2 * h + i + 1) * 128],
                                         U2[:, h * 256:(h + 1) * 256],
                                         start=True, stop=True)
                    Snew = spool.tile([128, 2 * DH], BF, tag=f"S{h}",
                                      name=f"S{h}")
                    nc.vector.tensor_tensor(Snew[:], S[h][:], ktds[:, 0:512],
                                            ALU.add)
                    S[h] = Snew
            return OhT2

        def ln_rows(src_ap, dst_rows, nrows):
            yr = lnp.tile([128, D], BF, tag="yr", name="yr")
            nc.gpsimd.dma_start(yr[0:nrows, :], src_ap)
            srow = lnp.tile([128, 1], F32, tag="srow", name="srow")
            nc.vector.tensor_reduce(srow[0:nrows, :], yr[0:nrows, :],
                                    mybir.AxisListType.X, ALU.add)
            ysq = lnp.tile([128, D], BF, tag="ysq", name="ysq")
            ssq = lnp.tile([128, 1], F32, tag="ssq", name="ssq")
            nc.scalar.activation(ysq[0:nrows, :], yr[0:nrows, :],
                                 ACTF.Square, accum_out=ssq[0:nrows, :])
            mneg = lnp.tile([128, 1], F32, tag="mneg", name="mneg")
            nc.scalar.mul(mneg[0:nrows, :], srow[0:nrows, :], -1.0 / D)
            mu2 = lnp.tile([128, 1], F32, tag="mu2", name="mu2")
            nc.vector.tensor_tensor(mu2[0:nrows, :], mneg[0:nrows, :],
                                    mneg[0:nrows, :], ALU.mult)
            var = lnp.tile([128, 1], F32, tag="var", name="var")
            nc.vector.scalar_tensor_tensor(var[0:nrows, :], ssq[0:nrows, :],
                                           1.0 / D, mu2[0:nrows, :], ALU.mult,
                                           ALU.subtract)
            sdv = lnp.tile([128, 1], F32, tag="sdv", name="sdv")
            nc.scalar.activation(sdv[0:nrows, :], var[0:nrows, :], ACTF.Sqrt,
                                 bias=EPSL[0:nrows, :])
            rstd = lnp.tile([128, 1], F32, tag="rstd", name="rstd")
            nc.vector.reciprocal_approx_fast(rstd[0:nrows, :], sdv[0:nrows, :])
            bcl = lnp.tile([128, 1], F32, tag="bcl", name="bcl")
            nc.vector.tensor_tensor(bcl[0:nrows, :], mneg[0:nrows, :],
                                    rstd[0:nrows, :], ALU.mult)
            yn = lnp.tile([128, D], BF, tag="yn", name="yn")
            nc.scalar.activation(yn[0:nrows, :], yr[0:nrows, :], ACTF.Identity,
                                 scale=rstd[0:nrows, :], bias=bcl[0:nrows, :])
            yg = lnp.tile([128, D], BF, tag="ysq", name="yg")
            nc.vector.tensor_tensor(yg[0:nrows, :], yn[0:nrows, :],
                                    LNG[0:nrows, :], ALU.mult)
            yfin = lnp.tile([128, D], BF, tag="yr", name="yfin")
            nc.vector.tensor_tensor(yfin[0:nrows, :], yg[0:nrows, :],
                                    LNB[0:nrows, :], ALU.add)
            nc.gpsimd.dma_start(y_out[dst_rows, :], yfin[0:nrows, :])

        def emit_ln(rb):
            yoff, half = ybase[rb]
            for r0 in range(0, half, 128):
                nr = min(128, half - r0)
                ln_rows(yhb[rb][r0:r0 + nr, :],
                        slice(yoff + r0, yoff + r0 + nr), nr)

        pending = []
        Pmap = {}
        for _c in range(min(3, n_ck)):
            Pmap[_c] = prep2(_c)
        for c in range(n_ck):
            cc = slice(c * CK, (c + 1) * CK)
            for rb, done_c in list(pending):
                defer = 3 if blocks[rb][1] <= 12 else 2
                if done_c <= c - defer:
                    emit_ln(rb)
                    pending.remove((rb, done_c))
            if c + 3 < n_ck:
                Pmap[c + 3] = prep2(c + 3)
            oht = spart2(c, Pmap.pop(c))
            # -- partial y = o @ Wo + 0.5x for this chunk --
            xc = xcp.tile([128, D], BF, tag="xc", name="xc")
            nc.sync.dma_start(xc[:], xtm[cc, :])
            for n in range(2):
                psy = ps_pay()
                for kk in range(4):
                    nc.tensor.matmul(psy[:], oht[:, kk * 128:(kk + 1) * 128],
                                     WO[kk][:, n * 512:(n + 1) * 512],
                                     start=(kk == 0), stop=(kk == 3))
                ysb = work.tile([128, 512], BF, tag="ysb", name="ysb")
                if n == 0:
                    nc.vector.tensor_tensor(ysb[:], psy[:],
                                            xc[:, n * 512:(n + 1) * 512],
                                            ALU.add)
                else:
                    nc.scalar.activation(ysb[:], psy[:], ACTF.Identity,
                                         bias=None, scale=1.0,
                                         accum_out=None) if False else                         nc.vector.tensor_tensor(ysb[:], psy[:],
                                                xc[:, n * 512:(n + 1) * 512],
                                                ALU.add)
                nc.sync.dma_start(ydr[c * CK:(c + 1) * CK, n * 512:(n + 1) * 512],
                                  ysb[:])
            # -- overlapped ReduceScatter; LayerNorm deferred 2 chunks --
            for rb, (s, e) in enumerate(blocks):
                if c + 1 == e:
                    nc.gpsimd.collective_compute(
                        "ReduceScatter", ALU.add,
                        replica_groups=[[0, 1], [2, 3], [4, 5], [6, 7]],
                        ins=[ydr[s * CK:e * CK, :]], outs=[yhb[rb].opt()],
                    )
                    pending.append((rb, c))
        for rb, done_c in pending:
            emit_ln(rb)

    nc.compile()
    return nc


def _shard(inputs, T=T_FULL):
    import ml_dtypes
    BFNP = ml_dtypes.bfloat16
    x = np.asarray(inputs["x"], dtype=np.float32)
    bf = lambda a: np.ascontiguousarray(np.asarray(a, dtype=np.float32)
                                        .astype(BFNP))
    f32 = lambda a: np.ascontiguousarray(np.asarray(a), dtype=np.float32)
    Wq, Wk, Wv = inputs["Wq"], inputs["Wk"], inputs["Wv"]
    Wb, Wo = inputs["Wb"], inputs["Wo"]
    cq, ck, cv = inputs["conv_q"], inputs["conv_k"], inputs["conv_v"]
    g_rms, ln_g, ln_b = (np.asarray(inputs["g_rms"], np.float32),
                         np.asarray(inputs["ln_g"], np.float32),
                         np.asarray(inputs["ln_b"], np.float32))

    ident = np.eye(128, dtype=np.float32)
    ii, jj = np.indices((128, 128))
    mlo = (jj < ii).astype(np.float32)
    mup = (jj > ii).astype(np.float32)
    mui = (jj >= ii).astype(np.float32)
    grms_col = np.tile(g_rms, 2)[:, None]  # [DG, 1] scales for Wo rows

    def conv_diag(cw, gs):
        """[128, 16*128]: per j-tile, diag(c0_j)..diag(c3_j)."""
        cg = np.asarray(cw, np.float32)[:, gs]  # [K, DG]
        blocks = []
        for j in range(4):
            for tap in range(4):
                blocks.append(np.diag(cg[tap, j * 128:(j + 1) * 128]))
        return np.concatenate(blocks, axis=1)

    in_maps = []
    for c in range(N_CORES):
        b, g = c // 2, c % 2
        gs = slice(g * DG, (g + 1) * DG)
        in_maps.append({
            "xt": bf(np.concatenate([np.zeros((D, 4), np.float32),
                                     x[b, :T].T], axis=1)),
            "xtm": bf(0.5 * x[b, :T]),
            "wq": bf(np.asarray(Wq)[:, gs]), "wk": bf(np.asarray(Wk)[:, gs]),
            "wv": bf(np.asarray(Wv)[:, gs]),
            "wb2": bf(np.asarray(Wb)[:, 2 * g:2 * g + 2]),
            "cqT": f32(np.asarray(cq)[:, gs].T),
            "ckT": f32(np.asarray(ck)[:, gs].T),
            "cvT": f32(np.asarray(cv)[:, gs].T),
            "cdq": bf(conv_diag(cq, gs)),
            "cdk": bf(conv_diag(ck, gs)),
            "cdv": bf(conv_diag(cv, gs)),
            "wo": bf(np.asarray(Wo)[gs, :] * grms_col),
            "lng": bf(np.tile(ln_g[None, :], (128, 1))),
            "lnb": bf(np.tile(ln_b[None, :], (128, 1))),
            "ident": bf(ident), "ones": bf(np.ones((128, 128), np.float32)),
            "id2f": f32(np.eye(2, dtype=np.float32)),
            "mlo": bf(mlo), "mup": bf(mup), "mui": bf(mui),
        })
    return in_maps


def kernel(**inputs):
    from concourse.bass_utils import run_bass_kernel_spmd
    T = T_FULL
    if "nc" not in _cache:
        _cache["nc"] = _build(T)
    nc = _cache["nc"]
    in_maps = _shard(inputs, T)
    res = run_bass_kernel_spmd(nc, in_maps, core_ids=list(range(N_CORES)))
    out = np.empty((B, T, D), dtype=np.float32)
    ridx = _rows_idx(T)
    for c in range(N_CORES):
        b, g = c // 2, c % 2
        out[b, ridx[g]] = np.asarray(res.results[c]["y_out"], dtype=np.float32)
    return out



# revision 46
# speedup vs baseline: 1.1484x; 1.1484x over previous
"""Trainium2 Bass kernel for the DeltaNet-style block nn_Block_68341519614809.

All-bf16 matmuls (1 cyc/row vs fp32-HIGH ~2.4), SBUF-resident q/k/v (no
DRAM staging), A^4-truncated triangular inverse (validated vs fp64 numpy:
8.3e-3 rel, tol 2e-2), scale folding (raw q with r_q folded into the RMS
scale, g_rms into Wo, -beta*r_k into KbTn), reciprocal_approx_fast for
rsqrt chains, x-residual via identity-matmul into the output-projection
psum (0.5x on each pair core), bf16 ReduceScatter in blocks of [4,4,4,2,2]
chunks with LayerNorm deferred 2 chunks (gpsimd-queue DMAs), bf16 y_out
(host casts to fp32).

Sharding: core c = 2*b + g  (b in 0..3 batch, g in 0..1 head-group of 2
heads).  ReduceScatter over core pairs; each core LayerNorms its
half-rows; host gathers.
"""
import numpy as np

B, T_FULL, D, H, CONV_K = 4, 2048, 1024, 4, 4
DH = 256          # head dim
DG = 512          # head-group width (2 heads)
EPS = 1e-5
CK = 128          # delta-rule chunk size
TB = 256          # stage-A t-block
N_CORES = 8

_cache = {}


def _rs_blocks(T):
    """(start_chunk, end_chunk) per ReduceScatter block; finer at the tail."""
    n_ck = T // CK
    if n_ck == 16:
        return [(0, 4), (4, 8), (8, 12), (12, 14), (14, 16)]
    return [(i, i + n_ck // 2) for i in range(0, n_ck, n_ck // 2)]


def _rows_idx(T):
    """Global t-rows owned by head-group g under the blocked ReduceScatter."""
    import numpy as _np
    segs = []
    for s, e in _rs_blocks(T):
        half = (e - s) * CK // 2
        segs.append((s * CK, half))
    out = {}
    for g in range(2):
        out[g] = _np.concatenate(
            [_np.arange(t0 + g * half, t0 + (g + 1) * half)
             for t0, half in segs])
    return out


def _build(T=T_FULL):
    from contextlib import ExitStack
    import concourse.bacc as bacc
    import concourse.tile as tile
    import concourse.mybir as mybir

    F32 = mybir.dt.float32
    BF = mybir.dt.bfloat16
    ALU = mybir.AluOpType
    ACTF = mybir.ActivationFunctionType

    n_tb = T // TB
    n_ck = T // CK
    TH = T // 2
    seg = TB + 4

    nc = bacc.Bacc("TRN2", target_bir_lowering=False, debug=False,
                   num_devices=N_CORES)

    # ---- I/O (all bf16 unless noted) ----
    xt = nc.dram_tensor("xt", [D, T + 4], BF, kind="ExternalInput")
    xtm = nc.dram_tensor("xtm", [T, D], BF, kind="ExternalInput")  # 0.5*x[b]
    wq = nc.dram_tensor("wq", [D, DG], BF, kind="ExternalInput")
    wk = nc.dram_tensor("wk", [D, DG], BF, kind="ExternalInput")
    wv = nc.dram_tensor("wv", [D, DG], BF, kind="ExternalInput")
    wb2 = nc.dram_tensor("wb2", [D, 2], BF, kind="ExternalInput")
    cqT = nc.dram_tensor("cqT", [DG, CONV_K], F32, kind="ExternalInput")
    ckT = nc.dram_tensor("ckT", [DG, CONV_K], F32, kind="ExternalInput")
    cvT = nc.dram_tensor("cvT", [DG, CONV_K], F32, kind="ExternalInput")
    # diag(c0..c3) per j-tile, for the PE-side conv taps
    cdq = nc.dram_tensor("cdq", [128, 16 * 128], BF, kind="ExternalInput")
    cdk = nc.dram_tensor("cdk", [128, 16 * 128], BF, kind="ExternalInput")
    cdv = nc.dram_tensor("cdv", [128, 16 * 128], BF, kind="ExternalInput")
    wo = nc.dram_tensor("wo", [DG, D], BF, kind="ExternalInput")  # g_rms folded
    lng = nc.dram_tensor("lng", [128, D], BF, kind="ExternalInput")
    lnb = nc.dram_tensor("lnb", [128, D], BF, kind="ExternalInput")
    ident_in = nc.dram_tensor("ident", [128, 128], BF, kind="ExternalInput")
    id2_in = nc.dram_tensor("id2f", [2, 2], F32, kind="ExternalInput")
    ones_in = nc.dram_tensor("ones", [128, 128], BF, kind="ExternalInput")
    mlo_in = nc.dram_tensor("mlo", [128, 128], BF, kind="ExternalInput")  # +(j<i)
    mup_in = nc.dram_tensor("mup", [128, 128], BF, kind="ExternalInput")  # +(j>i)
    mui_in = nc.dram_tensor("mui", [128, 128], BF, kind="ExternalInput")  # (j>=i)
    y_out = nc.dram_tensor("y_out", [TH, D], BF, kind="ExternalOutput")

    with tile.TileContext(nc) as tc, ExitStack() as top:
        top.enter_context(nc.allow_low_precision(
            reason="bf16 pipeline validated 8.3e-3 rel vs 2e-2 tol in sim"))
        const = top.enter_context(tc.tile_pool(name="const", bufs=1))
        psum = top.enter_context(tc.tile_pool(name="psum", bufs=2, space="PSUM"))
        dram = top.enter_context(tc.tile_pool(name="dram", bufs=1, space="DRAM"))

        def ps_pay():
            return psum.tile([128, 512], F32, tag="pay", name="pay")

        def ps_pg():
            return psum.tile([128, 256], F32, tag="pg", name="pg")

        def ps_med():
            return psum.tile([128, 512], F32, tag="pmed", name="pmed")

        def ps_small():
            return psum.tile([128, 512], F32, tag="psmall", name="psmall")

        # ---- stage-A weights + x first (startup critical path) ----
        wpool = top.enter_context(tc.tile_pool(name="wpool", bufs=1))
        xbp = top.enter_context(tc.tile_pool(name="xbp", bufs=2))
        WT = {}

        def load_w(nm, wsrc):
            wt = wpool.tile([128, 8 * DG], BF, tag=f"w{nm}", name=f"w{nm}")
            nc.sync.dma_start(wt[:].rearrange("p (k d) -> p k d", k=8),
                              wsrc[:].rearrange("(k p) d -> p k d", p=128))
            WT[nm] = wt

        def load_xb(t0):
            xb = xbp.tile([128, 8 * seg], BF, tag="xb", name="xb")
            nc.sync.dma_start(
                xb[:].rearrange("p (k t) -> p k t", k=8),
                xt[:].rearrange("(k p) t -> p k t", p=128)[:, :, t0:t0 + seg])
            return xb

        load_w("q", wq)
        xb0 = load_xb(0)
        WB2 = wpool.tile([128, 16], BF, tag="wb2", name="wb2")
        nc.sync.dma_start(WB2[:].rearrange("p (k j) -> p k j", k=8),
                          wb2[:].rearrange("(k p) j -> p k j", p=128))
        load_w("k", wk)
        load_w("v", wv)

        # ---- constants ----
        IB = const.tile([128, 128], BF, tag="ib", name="ib")
        nc.sync.dma_start(IB[:], ident_in[:])
        ONES = const.tile([128, 128], BF, tag="ones", name="ones")
        nc.sync.dma_start(ONES[:], ones_in[:])
        MLOP2 = const.tile([128, 256], BF, tag="mlop", name="mlop")
        nc.sync.dma_start(MLOP2[:, 0:128], mlo_in[:])
        nc.sync.dma_start(MLOP2[:, 128:256], mlo_in[:])
        MUPP2 = const.tile([128, 256], BF, tag="mupp", name="mupp")
        nc.sync.dma_start(MUPP2[:, 0:128], mup_in[:])
        nc.sync.dma_start(MUPP2[:, 128:256], mup_in[:])
        MUI2 = const.tile([128, 256], BF, tag="mui", name="mui")
        nc.sync.dma_start(MUI2[:, 0:128], mui_in[:])
        nc.sync.dma_start(MUI2[:, 128:256], mui_in[:])
        EPS1 = const.tile([128, 1], F32, tag="eps1", name="eps1")
        nc.gpsimd.memset(EPS1[:], 1e-6)
        EPSL = const.tile([128, 1], F32, tag="epsl", name="epsl")
        nc.gpsimd.memset(EPSL[:], EPS)
        ONE32 = const.tile([1, 1], F32, tag="one32", name="one32")
        nc.gpsimd.memset(ONE32[:], 1.0)
        ID2 = const.tile([2, 2], F32, tag="id2", name="id2")
        nc.sync.dma_start(ID2[:], id2_in[:])
        CD = {}
        for nm, cd in (("q", cdq), ("k", cdk), ("v", cdv)):
            CD[nm] = const.tile([128, 16 * 128], BF, tag=f"cd{nm}", name=f"cd{nm}")
            nc.sync.dma_start(CD[nm][:], cd[:])

        # beta rows + q/k/v stay in SBUF (bf16, d-major, col = j*T + t)
        BT = [const.tile([1, T], F32, tag=f"BT{h}", name=f"BT{h}") for h in range(2)]
        QKV = {}
        for nm in ("q", "k", "v"):
            QKV[nm] = const.tile([128, 4 * T], BF, tag=f"qkv{nm}", name=f"qkv{nm}")

        # ================= stage A: projections + conv + silu =================
        cvp = top.enter_context(tc.tile_pool(name="cvp", bufs=3))

        def conv_tail(st):
            """all 4 conv taps as diag-matmuls on PE; silu straight from PSUM."""
            nm, j, t0, u = st
            ps2 = ps_pg()
            for tap in range(4):
                nc.tensor.matmul(ps2[:, 0:TB],
                                 CD[nm][:, (4 * j + tap) * 128:
                                           (4 * j + tap + 1) * 128],
                                 u[:, 1 + tap:TB + 1 + tap],
                                 start=(tap == 0), stop=(tap == 3))
            nc.scalar.activation(QKV[nm][:, j * T + t0:j * T + t0 + TB],
                                 ps2[:, 0:TB], ACTF.Silu)

        pend_conv = []

        def emit_tb(tb):
            t0 = tb * TB
            xb = xb0 if tb == 0 else load_xb(t0)
            for h in range(2):
                psb = ps_small()
                for k in range(8):
                    nc.tensor.matmul(psb[0:1, 0:TB],
                                     WB2[:, k * 2 + h:k * 2 + h + 1],
                                     xb[:, k * seg + 4:(k + 1) * seg],
                                     start=(k == 0), stop=(k == 7))
                bth = cvp.tile([1, TB], F32, tag="bth", name="bth")
                nc.scalar.activation(bth[0:1, 0:TB], psb[0:1, 0:TB],
                                     ACTF.Tanh, scale=0.5)
                nc.vector.tensor_scalar(BT[h][0:1, t0:t0 + TB], bth[0:1, 0:TB],
                                        0.5, 0.5, ALU.mult, ALU.add)

            for nm in ("q", "k", "v"):
                for j in range(4):
                    ps = ps_pay()
                    for k in range(8):
                        nc.tensor.matmul(ps[:, 0:seg],
                                         WT[nm][:, k * DG + j * 128:
                                                    k * DG + (j + 1) * 128],
                                         xb[:, k * seg:(k + 1) * seg],
                                         start=(k == 0), stop=(k == 7))
                    # conv[t] = sum_i cw[i]*pre[t-3+i]; ps col (t-t0+4)
                    u = cvp.tile([128, seg], BF, tag="u", name="u")
                    nc.vector.tensor_copy(u[:, 1:seg], ps[:, 1:seg])
                    if pend_conv:
                        conv_tail(pend_conv.pop())
                    pend_conv.append((nm, j, t0, u))

        for _tb in range(n_tb):
            emit_tb(_tb)
        while pend_conv:
            conv_tail(pend_conv.pop())

        # ================= chunk stage: delta rule =================
        ckx = top.enter_context(ExitStack())
        work = ckx.enter_context(tc.tile_pool(name="work", bufs=3))
        spool = ckx.enter_context(tc.tile_pool(name="spool", bufs=2))
        ohp = ckx.enter_context(tc.tile_pool(name="ohp", bufs=2))
        wop = ckx.enter_context(tc.tile_pool(name="wop", bufs=1))
        xcp = ckx.enter_context(tc.tile_pool(name="xcp", bufs=2))

        WO = [wop.tile([128, D], BF, tag=f"wo{k}", name=f"wo{k}") for k in range(4)]
        for k in range(4):
            nc.sync.dma_start(WO[k][:], wo[k * 128:(k + 1) * 128, :])

        ydr = dram.tile([T, D], BF, tag="ydr", name="ydr")

        S = {}
        for h in range(2):
            S[h] = spool.tile([128, 2 * DH], BF, tag=f"S{h}", name=f"S{h}")
            nc.gpsimd.memset(S[h][:], 0.0)

        blocks = _rs_blocks(T)
        ybase = []
        off = 0
        for s, e in blocks:
            half = (e - s) * CK // 2
            ybase.append((off, half))
            off += half
        yhb = [dram.tile([ybase[rb][1], D], BF, tag=f"yhb{rb}", name=f"yhb{rb}")
               for rb in range(len(blocks))]

        LNG = const.tile([128, D], BF, tag="lng", name="lng")
        nc.sync.dma_start(LNG[:], lng[:])
        LNB = const.tile([128, D], BF, tag="lnb", name="lnb")
        nc.sync.dma_start(LNB[:], lnb[:])
        lnp = ckx.enter_context(tc.tile_pool(name="lnp", bufs=2))

        def dt_ap(nm, h, i, cc):
            """single d-tile [128, 128]"""
            j = 2 * h + i
            return QKV[nm][:, j * T + cc.start:j * T + cc.stop]

        def prep2(c):
            """Per-chunk prep for BOTH heads; raw-k Gram with scales folded
            into per-partition columns (CB2) and row-broadcast masks."""
            cc = slice(c * CK, (c + 1) * CK)

            # -- l2 norm sums: SQ2 [q0|k0|q1|k1|q2|k2|q3|k3] (j = 2h+i) --
            SQ2 = work.tile([128, 1024], BF, tag="SQ", name="SQ", bufs=2)
            sqw = SQ2[:].rearrange("p (j t) -> p j t", j=4)
            qa = QKV["q"][:].rearrange("p (j t) -> p j t", j=4)[:, :, cc]
            ka = QKV["k"][:].rearrange("p (j t) -> p j t", j=4)[:, :, cc]
            nc.vector.tensor_tensor(sqw[:, :, 0:128], qa, qa, ALU.mult)
            nc.vector.tensor_tensor(sqw[:, :, 128:256], ka, ka, ALU.mult)
            psn = ps_small()  # [1,512] = [nq0|nk0|nq1|nk1]
            for h in range(2):
                for i in range(2):
                    nc.tensor.matmul(psn[0:1, h * 256:(h + 1) * 256],
                                     ONES[:, 0:1],
                                     SQ2[:, (2 * h + i) * 256:
                                            (2 * h + i + 1) * 256],
                                     start=(i == 0), stop=(i == 1))
            sqr = work.tile([1, 512], F32, tag="sqr", name="sqr")
            nc.scalar.activation(sqr[:], psn[0:1, 0:512], ACTF.Sqrt,
                                 bias=EPS1[0:1, 0:1])
            R3f = work.tile([1, 512], F32, tag="R3f", name="R3f")
            nc.vector.reciprocal_approx_fast(R3f[:], sqr[:])
            # RCAT [1,512] = [rk0|mb0|rk1|mb1]  (mb = -beta*rk)
            RCAT = work.tile([1, 512], BF, tag="RCAT", name="RCAT")
            rcw = RCAT[0:1].rearrange("p (i t) -> p i t", i=2)
            r3w = R3f[0:1].rearrange("p (i t) -> p i t", i=2)
            nc.vector.tensor_copy(rcw[:, :, 0:128], r3w[:, :, 128:256])
            for h in range(2):
                nc.vector.scalar_tensor_tensor(
                    RCAT[0:1, h * 256 + 128:(h + 1) * 256],
                    R3f[0:1, h * 256 + 128:(h + 1) * 256],
                    -1.0, BT[h][0:1, cc], ALU.mult, ALU.mult)
            psbr = ps_small()
            nc.tensor.matmul(psbr[0:128, 0:512], ONES[0:1, :], RCAT[:],
                             start=True, stop=True)
            RB = work.tile([128, 512], BF, tag="RB", name="RB", bufs=2)
            nc.scalar.activation(RB[:], psbr[0:128, 0:512], ACTF.Copy)
            # columns: CB2 [128,6]: per h: rq(3h) b(3h+1) rk(3h+2); MBC: mb
            pst1 = ps_small()
            for h in range(2):
                nc.tensor.transpose(pst1[0:128, 3 * h:3 * h + 1],
                                    R3f[0:1, h * 256:h * 256 + 128],
                                    ONE32[0:1, 0:1])
                nc.tensor.transpose(pst1[0:128, 3 * h + 1:3 * h + 2],
                                    BT[h][0:1, cc], ONE32[0:1, 0:1])
                nc.tensor.transpose(pst1[0:128, 3 * h + 2:3 * h + 3],
                                    R3f[0:1, h * 256 + 128:(h + 1) * 256],
                                    ONE32[0:1, 0:1])
            pstb = pst1[:, 6:8].bitcast(BF)  # [128, 4] bf16 view
            for h in range(2):
                nc.tensor.transpose(pstb[0:128, 2 * h:2 * h + 1],
                                    RCAT[0:1, h * 256 + 128:(h + 1) * 256],
                                    ONES[0:1, 0:1])
            CB2 = work.tile([128, 6], F32, tag="CB", name="CB", bufs=6)
            nc.scalar.copy(CB2[:], pst1[0:128, 0:6])
            MBC = work.tile([128, 2], F32, tag="MBC", name="MBC", bufs=6)
            nc.vector.tensor_copy(
                MBC[:].rearrange("p (h x) -> p h x", x=1),
                pstb[0:128].rearrange("p (h x) -> p h x", h=2)[:, :, 0:1])
            cbh = CB2[:].rearrange("p (h x) -> p h x", h=2)
            r2dh2 = work.tile([128, 2], F32, tag="r2dh", name="r2dh", bufs=6)
            nc.vector.scalar_tensor_tensor(
                r2dh2[:].rearrange("p (h x) -> p h x", x=1),
                cbh[:, :, 0:1], 1.0 / DH, cbh[:, :, 0:1], ALU.mult, ALU.mult)

            # -- row-broadcast masks: MLORK = mlo*rk_rows, MUPBK = mup*mb_rows
            rb2 = RB[:].rearrange("p (i t) -> p i t", i=2)
            MLORK = work.tile([128, 256], BF, tag="MLORK", name="MLORK", bufs=2)
            nc.vector.tensor_tensor(
                MLORK[:].rearrange("p (i t) -> p i t", i=2),
                MLOP2[:].rearrange("p (i t) -> p i t", i=2),
                rb2[:, :, 0:128], ALU.mult)
            MUPBK = work.tile([128, 256], BF, tag="MUPBK", name="MUPBK", bufs=2)
            nc.vector.tensor_tensor(
                MUPBK[:].rearrange("p (i t) -> p i t", i=2),
                MUPP2[:].rearrange("p (i t) -> p i t", i=2),
                rb2[:, :, 128:256], ALU.mult)

            # -- Gram (symmetric, raw k): psg [G_h0 | G_h1] --
            psg = ps_pg()
            for h in range(2):
                for i in range(2):
                    nc.tensor.matmul(psg[:, h * 128:(h + 1) * 128],
                                     dt_ap("k", h, i, cc),
                                     dt_ap("k", h, i, cc),
                                     start=(i == 0), stop=(i == 1))
            An2 = work.tile([128, 256], BF, tag="An", name="An", bufs=2)
            ATn2 = work.tile([128, 256], BF, tag="ATn", name="ATn", bufs=2)
            for h in range(2):
                sl = slice(h * 128, (h + 1) * 128)
                nc.vector.scalar_tensor_tensor(An2[:, sl], psg[:, sl],
                                               MBC[:, h:h + 1],
                                               MLORK[:, sl], ALU.mult, ALU.mult)
                nc.vector.scalar_tensor_tensor(ATn2[:, sl], psg[:, sl],
                                               CB2[:, 3 * h + 2:3 * h + 3],
                                               MUPBK[:, sl], ALU.mult, ALU.mult)

            # -- truncated inverse transpose: TpT = (I+A4T)(I+A2T)(I-AT) --
            pp = ps_pay()  # [P2_0|P2T_0|P2_1|P2T_1]
            for h in range(2):
                sl = slice(h * 128, (h + 1) * 128)
                nc.tensor.matmul(pp[:, h * 256:h * 256 + 128], ATn2[:, sl],
                                 An2[:, sl], start=True, stop=True)
                nc.tensor.matmul(pp[:, h * 256 + 128:(h + 1) * 256],
                                 An2[:, sl], ATn2[:, sl], start=True, stop=True)
            PP = work.tile([128, 512], BF, tag="PP", name="PP", bufs=2)
            nc.scalar.copy(PP[:], pp[:, 0:512])
            pr1 = ps_pg()
            for h in range(2):
                sl = slice(h * 128, (h + 1) * 128)
                nc.tensor.matmul(pr1[:, sl], PP[:, h * 256:h * 256 + 128],
                                 PP[:, h * 256 + 128:(h + 1) * 256],
                                 start=True, stop=False)
                nc.tensor.matmul(pr1[:, sl], IB[:], IB[:], start=False,
                                 stop=True)
            R1 = work.tile([128, 256], BF, tag="R1", name="R1", bufs=2)
            nc.scalar.copy(R1[:], pr1[:, 0:256])
            pr2 = ps_pg()
            for h in range(2):
                sl = slice(h * 128, (h + 1) * 128)
                nc.tensor.matmul(pr2[:, sl], PP[:, h * 256:h * 256 + 128],
                                 R1[:, sl], start=True, stop=False)
                nc.tensor.matmul(pr2[:, sl], IB[:], R1[:, sl], start=False,
                                 stop=True)
            R2 = work.tile([128, 256], BF, tag="R2", name="R2", bufs=2)
            nc.scalar.copy(R2[:], pr2[:, 0:256])
            pr3 = ps_pg()
            for h in range(2):
                sl = slice(h * 128, (h + 1) * 128)
                nc.tensor.matmul(pr3[:, sl], An2[:, sl], R2[:, sl],
                                 start=True, stop=False)
                nc.tensor.matmul(pr3[:, sl], IB[:], R2[:, sl], start=False,
                                 stop=True)
            TTt2 = work.tile([128, 256], BF, tag="TTt", name="TTt", bufs=5)
            nc.scalar.copy(TTt2[:], pr3[:, 0:256])

            # -- MT' = triu(k_raw^T q_raw) (rk rides on U') --
            psmt = ps_pg()
            for h in range(2):
                for i in range(2):
                    nc.tensor.matmul(psmt[:, h * 128:(h + 1) * 128],
                                     dt_ap("k", h, i, cc),
                                     dt_ap("q", h, i, cc),
                                     start=(i == 0), stop=(i == 1))
            MT2 = work.tile([128, 256], BF, tag="MT", name="MT", bufs=5)
            nc.vector.tensor_tensor(MT2[:], psmt[:, 0:256], MUI2[:], ALU.mult)

            # -- beta*V (t-major) via PE transpose --
            vw = ps_med()
            vwb = vw[:, 0:256].bitcast(BF)  # [128, 512] bf16 view
            for h in range(2):
                for i in range(2):
                    nc.tensor.transpose(vwb[:, (2 * h + i) * 128:
                                               (2 * h + i + 1) * 128],
                                        dt_ap("v", h, i, cc), IB[:])
            Vtb2 = work.tile([128, 512], BF, tag="Vtb", name="Vtb", bufs=5)
            for h in range(2):
                nc.scalar.activation(Vtb2[:, h * 256:(h + 1) * 256],
                                     vwb[:, h * 256:(h + 1) * 256],
                                     ACTF.Copy, scale=CB2[:, 3 * h + 1:3 * h + 2])

            # -- raw K t-major (rk rides on U') --
            pskt = ps_med()
            psktb = pskt[:, 0:256].bitcast(BF)
            for h in range(2):
                for i in range(2):
                    nc.tensor.transpose(psktb[:, (2 * h + i) * 128:
                                                (2 * h + i + 1) * 128],
                                        dt_ap("k", h, i, cc), IB[:])
            Kh2 = work.tile([128, 512], BF, tag="Kh", name="Kh", bufs=5)
            nc.vector.tensor_copy(Kh2[:], psktb[:])

            return dict(TTt=TTt2, Vtb=Vtb2, MT=MT2, Kh=Kh2, CB=CB2,
                        MBC=MBC, r2dh=r2dh2)

        def spart2(c, Pd):
            cc = slice(c * CK, (c + 1) * CK)
            TTt2, Vtb2, MT2 = Pd["TTt"], Pd["Vtb"], Pd["MT"]
            Kh2, CB2, r2dh2 = Pd["Kh"], Pd["CB"], Pd["r2dh"]
            MBC = Pd["MBC"]

            # kS = k_raw^T S; W2b = (kS * mb) + beta*Vt  (mb = -beta*rk)
            psw = ps_med()
            for h in range(2):
                for i in range(2):
                    nc.tensor.matmul(psw[:, h * 256:(h + 1) * 256],
                                     dt_ap("k", h, i, cc),
                                     S[h][:, i * 256:(i + 1) * 256],
                                     start=(i == 0), stop=(i == 1))
            W2b2 = work.tile([128, 512], BF, tag="W2b", name="W2b", bufs=2)
            for h in range(2):
                sl = slice(h * 256, (h + 1) * 256)
                nc.vector.scalar_tensor_tensor(W2b2[:, sl], psw[:, sl],
                                               MBC[:, h:h + 1],
                                               Vtb2[:, sl], ALU.mult, ALU.add)
            # U' = rk * (Tp' W2b)
            pu = ps_med()
            for h in range(2):
                sl = slice(h * 256, (h + 1) * 256)
                nc.tensor.matmul(pu[:, sl], TTt2[:, h * 128:(h + 1) * 128],
                                 W2b2[:, sl], start=True, stop=True)
            U2 = work.tile([128, 512], BF, tag="U", name="U", bufs=2)
            for h in range(2):
                sl = slice(h * 256, (h + 1) * 256)
                nc.scalar.activation(U2[:, sl], pu[:, sl], ACTF.Copy,
                                     scale=CB2[:, 3 * h + 2:3 * h + 3])

            # O_raw (t-major) = q_raw S + MT' U'; per-head RMS, r_q folded
            pso = ps_med()
            for h in range(2):
                sl = slice(h * 256, (h + 1) * 256)
                for i in range(2):
                    nc.tensor.matmul(pso[:, sl], dt_ap("q", h, i, cc),
                                     S[h][:, i * 256:(i + 1) * 256],
                                     start=(i == 0), stop=False)
                nc.tensor.matmul(pso[:, sl], MT2[:, h * 128:(h + 1) * 128],
                                 U2[:, sl], start=False, stop=True)
            waste = work.tile([128, 512], BF, tag="waste", name="waste", bufs=1)
            sso2 = work.tile([128, 2], F32, tag="sso", name="sso")
            for h in range(2):
                nc.scalar.activation(waste[:, h * 256:(h + 1) * 256],
                                     pso[:, h * 256:(h + 1) * 256],
                                     ACTF.Square, accum_out=sso2[:, h:h + 1])
            ssp = work.tile([128, 2], F32, tag="ssp", name="ssp")
            nc.vector.tensor_tensor(ssp[:], sso2[:], r2dh2[:], ALU.mult)
            sdo = work.tile([128, 2], F32, tag="sdo", name="sdo")
            nc.scalar.activation(sdo[:], ssp[:], ACTF.Sqrt, bias=EPSL[:])
            rcoi = work.tile([128, 2], F32, tag="rcoi", name="rcoi")
            nc.vector.reciprocal_approx_fast(rcoi[:], sdo[:])
            rco = work.tile([128, 2], F32, tag="rco", name="rco")
            cbh = CB2[:].rearrange("p (h x) -> p h x", h=2)
            nc.vector.tensor_tensor(
                rco[:].rearrange("p (h x) -> p h x", x=1),
                rcoi[:].rearrange("p (h x) -> p h x", x=1),
                cbh[:, :, 0:1], ALU.mult)
            Ohn2 = work.tile([128, 512], BF, tag="Ohn", name="Ohn", bufs=2)
            for h in range(2):
                sl = slice(h * 256, (h + 1) * 256)
                nc.scalar.activation(Ohn2[:, sl], pso[:, sl], ACTF.Copy,
                                     scale=rco[:, h:h + 1])
            psot = ps_pg()
            psob = psot[:, 0:256].bitcast(BF)  # [128, 512] bf16 view
            for jj in range(4):
                nc.tensor.transpose(psob[:, jj * 128:(jj + 1) * 128],
                                    Ohn2[:, jj * 128:(jj + 1) * 128], IB[:])
            OhT2 = ohp.tile([128, 512], BF, tag="OhT", name="OhT")
            nc.scalar.copy(OhT2[:], psob[:])

            # S += k_raw^T U'  (skip on the final chunk)
            if c + 1 < n_ck:
                for h in range(2):
                    ktds = ps_med() if h == 0 else ps_pay()
                    for i in range(2):
                        reg = slice(i * 256, (i + 1) * 256)
                        nc.tensor.matmul(ktds[:, reg],
                                         Kh2[:, (2 * h + i) * 128:
                                                (2 * h + i + 1) * 128],
                                         U2[:, h * 256:(h + 1) * 256],
                                         start=True, stop=True)
                    Snew = spool.tile([128, 2 * DH], BF, tag=f"S{h}",
                                      name=f"S{h}")
                    nc.vector.tensor_tensor(Snew[:], S[h][:], ktds[:, 0:512],
                                            ALU.add)
                    S[h] = Snew
            return OhT2

        def ln_rows(src_ap, dst_rows, nrows):
            yr = lnp.tile([128, D], BF, tag="yr", name="yr")
            nc.gpsimd.dma_start(yr[0:nrows, :], src_ap)
            srow = lnp.tile([128, 1], F32, tag="srow", name="srow")
            nc.vector.tensor_reduce(srow[0:nrows, :], yr[0:nrows, :],
                                    mybir.AxisListType.X, ALU.add)
            ysq = lnp.tile([128, D], BF, tag="ysq", name="ysq")
            ssq = lnp.tile([128, 1], F32, tag="ssq", name="ssq")
            nc.scalar.activation(ysq[0:nrows, :], yr[0:nrows, :],
                                 ACTF.Square, accum_out=ssq[0:nrows, :])
            mneg = lnp.tile([128, 1], F32, tag="mneg", name="mneg")
            nc.scalar.mul(mneg[0:nrows, :], srow[0:nrows, :], -1.0 / D)
            mu2 = lnp.tile([128, 1], F32, tag="mu2", name="mu2")
            nc.vector.tensor_tensor(mu2[0:nrows, :], mneg[0:nrows, :],
                                    mneg[0:nrows, :], ALU.mult)
            var = lnp.tile([128, 1], F32, tag="var", name="var")
            nc.vector.scalar_tensor_tensor(var[0:nrows, :], ssq[0:nrows, :],
                                           1.0 / D, mu2[0:nrows, :], ALU.mult,
                                           ALU.subtract)
            sdv = lnp.tile([128, 1], F32, tag="sdv", name="sdv")
            nc.scalar.activation(sdv[0:nrows, :], var[0:nrows, :], ACTF.Sqrt,
                                 bias=EPSL[0:nrows, :])
            rstd = lnp.tile([128, 1], F32, tag="rstd", name="rstd")
            nc.vector.reciprocal_approx_fast(rstd[0:nrows, :], sdv[0:nrows, :])
            bcl = lnp.tile([128, 1], F32, tag="bcl", name="bcl")
            nc.vector.tensor_tensor(bcl[0:nrows, :], mneg[0:nrows, :],
                                    rstd[0:nrows, :], ALU.mult)
            yn = lnp.tile([128, D], BF, tag="yn", name="yn")
            nc.scalar.activation(yn[0:nrows, :], yr[0:nrows, :], ACTF.Identity,
                                 scale=rstd[0:nrows, :], bias=bcl[0:nrows, :])
            yg = lnp.tile([128, D], BF, tag="ysq", name="yg")
            nc.vector.tensor_tensor(yg[0:nrows, :], yn[0:nrows, :],
                                    LNG[0:nrows, :], ALU.mult)
            yfin = lnp.tile([128, D], BF, tag="yr", name="yfin")
            nc.vector.tensor_tensor(yfin[0:nrows, :], yg[0:nrows, :],
                                    LNB[0:nrows, :], ALU.add)
            nc.gpsimd.dma_start(y_out[dst_rows, :], yfin[0:nrows, :])

        def emit_ln(rb):
            yoff, half = ybase[rb]
            for r0 in range(0, half, 128):
                nr = min(128, half - r0)
                ln_rows(yhb[rb][r0:r0 + nr, :],
                        slice(yoff + r0, yoff + r0 + nr), nr)

        pending = []
        Pmap = {}
        for _c in range(min(3, n_ck)):
            Pmap[_c] = prep2(_c)
        for c in range(n_ck):
            cc = slice(c * CK, (c + 1) * CK)
            for rb, done_c in list(pending):
                defer = 3 if blocks[rb][1] <= 12 else 2
                if done_c <= c - defer:
                    emit_ln(rb)
                    pending.remove((rb, done_c))
            if c + 3 < n_ck:
                Pmap[c + 3] = prep2(c + 3)
            oht = spart2(c, Pmap.pop(c))
            # -- partial y = o @ Wo + 0.5x for this chunk --
            xc = xcp.tile([128, D], BF, tag="xc", name="xc")
            nc.sync.dma_start(xc[:], xtm[cc, :])
            for n in range(2):
                psy = ps_pay()
                for kk in range(4):
                    nc.tensor.matmul(psy[:], oht[:, kk * 128:(kk + 1) * 128],
                                     WO[kk][:, n * 512:(n + 1) * 512],
                                     start=(kk == 0), stop=(kk == 3))
                ysb = work.tile([128, 512], BF, tag="ysb", name="ysb")
                if n == 0:
                    nc.vector.tensor_tensor(ysb[:], psy[:],
                                            xc[:, n * 512:(n + 1) * 512],
                                            ALU.add)
                else:
                    nc.scalar.activation(ysb[:], psy[:], ACTF.Identity,
                                         bias=None, scale=1.0,
                                         accum_out=None) if False else                         nc.vector.tensor_tensor(ysb[:], psy[:],
                                                xc[:, n * 512:(n + 1) * 512],
                                                ALU.add)
                nc.sync.dma_start(ydr[c * CK:(c + 1) * CK, n * 512:(n + 1) * 512],
                                  ysb[:])
            # -- overlapped ReduceScatter; LayerNorm deferred 2 chunks --
            for rb, (s, e) in enumerate(blocks):
                if c + 1 == e:
                    nc.gpsimd.collective_compute(
                        "ReduceScatter", ALU.add,
                        replica_groups=[[0, 1], [2, 3], [4, 5], [6, 7]],
                        ins=[ydr[s * CK:e * CK, :]], outs=[yhb[rb].opt()],
                    )
                    pending.append((rb, c))
        for rb, done_c in pending:
            emit_ln(rb)

    nc.compile()
    return nc


def _shard(inputs, T=T_FULL):
    import ml_dtypes
    BFNP = ml_dtypes.bfloat16
    x = np.asarray(inputs["x"], dtype=np.float32)
    bf = lambda a: np.ascontiguousarray(np.asarray(a, dtype=np.float32)
                                        .astype(BFNP))
    f32 = lambda a: np.ascontiguousarray(np.asarray(a), dtype=np.float32)
    Wq, Wk, Wv = inputs["Wq"], inputs["Wk"], inputs["Wv"]
    Wb, Wo = inputs["Wb"], inputs["Wo"]
    cq, ck, cv = inputs["conv_q"], inputs["conv_k"], inputs["conv_v"]
    g_rms, ln_g, ln_b = (np.asarray(inputs["g_rms"], np.float32),
                         np.asarray(inputs["ln_g"], np.float32),
                         np.asarray(inputs["ln_b"], np.float32))

    ident = np.eye(128, dtype=np.float32)
    ii, jj = np.indices((128, 128))
    mlo = (jj < ii).astype(np.float32)
    mup = (jj > ii).astype(np.float32)
    mui = (jj >= ii).astype(np.float32)
    grms_col = np.tile(g_rms, 2)[:, None]  # [DG, 1] scales for Wo rows

    def conv_diag(cw, gs):
        """[128, 16*128]: per j-tile, diag(c0_j)..diag(c3_j)."""
        cg = np.asarray(cw, np.float32)[:, gs]  # [K, DG]
        blocks = []
        for j in range(4):
            for tap in range(4):
                blocks.append(np.diag(cg[tap, j * 128:(j + 1) * 128]))
        return np.concatenate(blocks, axis=1)

    in_maps = []
    for c in range(N_CORES):
        b, g = c // 2, c % 2
        gs = slice(g * DG, (g + 1) * DG)
        in_maps.append({
            "xt": bf(np.concatenate([np.zeros((D, 4), np.float32),
                                     x[b, :T].T], axis=1)),
            "xtm": bf(0.5 * x[b, :T]),
            "wq": bf(np.asarray(Wq)[:, gs]), "wk": bf(np.asarray(Wk)[:, gs]),
            "wv": bf(np.asarray(Wv)[:, gs]),
            "wb2": bf(np.asarray(Wb)[:, 2 * g:2 * g + 2]),
            "cqT": f32(np.asarray(cq)[:, gs].T),
            "ckT": f32(np.asarray(ck)[:, gs].T),
            "cvT": f32(np.asarray(cv)[:, gs].T),
            "cdq": bf(conv_diag(cq, gs)),
            "cdk": bf(conv_diag(ck, gs)),
            "cdv": bf(conv_diag(cv, gs)),
            "wo": bf(np.asarray(Wo)[gs, :] * grms_col),
            "lng": bf(np.tile(ln_g[None, :], (128, 1))),
            "lnb": bf(np.tile(ln_b[None, :], (128, 1))),
            "ident": bf(ident), "ones": bf(np.ones((128, 128), np.float32)),
            "id2f": f32(np.eye(2, dtype=np.float32)),
            "mlo": bf(mlo), "mup": bf(mup), "mui": bf(mui),
        })
    return in_maps


def kernel(**inputs):
    from concourse.bass_utils import run_bass_kernel_spmd
    T = T_FULL
    if "nc" not in _cache:
        _cache["nc"] = _build(T)
    nc = _cache["nc"]
    in_maps = _shard(inputs, T)
    res = run_bass_kernel_spmd(nc, in_maps, core_ids=list(range(N_CORES)))
    out = np.empty((B, T, D), dtype=np.float32)
    ridx = _rows_idx(T)
    for c in range(N_CORES):
        b, g = c // 2, c % 2
        out[b, ridx[g]] = np.asarray(res.results[c]["y_out"], dtype=np.float32)
    return out



# revision 47
# speedup vs baseline: 1.1962x; 1.0416x over previous
"""Trainium2 Bass kernel for the DeltaNet-style block nn_Block_68341519614809.

All-bf16 matmuls (1 cyc/row vs fp32-HIGH ~2.4), SBUF-resident q/k/v (no
DRAM staging), A^4-truncated triangular inverse (validated vs fp64 numpy:
8.3e-3 rel, tol 2e-2), scale folding (raw q with r_q folded into the RMS
scale, g_rms into Wo, -beta*r_k into KbTn), reciprocal_approx_fast for
rsqrt chains, x-residual via identity-matmul into the output-projection
psum (0.5x on each pair core), bf16 ReduceScatter in blocks of [4,4,4,2,2]
chunks with LayerNorm deferred 2 chunks (gpsimd-queue DMAs), bf16 y_out
(host casts to fp32).

Sharding: core c = 2*b + g  (b in 0..3 batch, g in 0..1 head-group of 2
heads).  ReduceScatter over core pairs; each core LayerNorms its
half-rows; host gathers.
"""
import numpy as np

B, T_FULL, D, H, CONV_K = 4, 2048, 1024, 4, 4
DH = 256          # head dim
DG = 512          # head-group width (2 heads)
EPS = 1e-5
CK = 128          # delta-rule chunk size
TB = 256          # stage-A t-block
N_CORES = 8

_cache = {}


def _rs_blocks(T):
    """(start_chunk, end_chunk) per ReduceScatter block; finer at the tail."""
    n_ck = T // CK
    if n_ck == 16:
        return [(0, 4), (4, 8), (8, 12), (12, 14), (14, 16)]
    return [(i, i + n_ck // 2) for i in range(0, n_ck, n_ck // 2)]


def _rows_idx(T):
    """Global t-rows owned by head-group g under the blocked ReduceScatter."""
    import numpy as _np
    segs = []
    for s, e in _rs_blocks(T):
        half = (e - s) * CK // 2
        segs.append((s * CK, half))
    out = {}
    for g in range(2):
        out[g] = _np.concatenate(
            [_np.arange(t0 + g * half, t0 + (g + 1) * half)
             for t0, half in segs])
    return out


def _build(T=T_FULL):
    from contextlib import ExitStack
    import concourse.bacc as bacc
    import concourse.tile as tile
    import concourse.mybir as mybir

    F32 = mybir.dt.float32
    BF = mybir.dt.bfloat16
    ALU = mybir.AluOpType
    ACTF = mybir.ActivationFunctionType

    n_tb = T // TB
    n_ck = T // CK
    TH = T // 2
    seg = TB + 4

    nc = bacc.Bacc("TRN2", target_bir_lowering=False, debug=False,
                   num_devices=N_CORES)

    # ---- I/O (all bf16 unless noted) ----
    xt = nc.dram_tensor("xt", [D, T + 4], BF, kind="ExternalInput")
    xtm = nc.dram_tensor("xtm", [T, D], BF, kind="ExternalInput")  # 0.5*x[b]
    wq = nc.dram_tensor("wq", [D, DG], BF, kind="ExternalInput")
    wk = nc.dram_tensor("wk", [D, DG], BF, kind="ExternalInput")
    wv = nc.dram_tensor("wv", [D, DG], BF, kind="ExternalInput")
    wb2 = nc.dram_tensor("wb2", [D, 2], BF, kind="ExternalInput")
    cqT = nc.dram_tensor("cqT", [DG, CONV_K], F32, kind="ExternalInput")
    ckT = nc.dram_tensor("ckT", [DG, CONV_K], F32, kind="ExternalInput")
    cvT = nc.dram_tensor("cvT", [DG, CONV_K], F32, kind="ExternalInput")
    # diag(c0..c3) per j-tile, for the PE-side conv taps
    cdq = nc.dram_tensor("cdq", [128, 16 * 128], BF, kind="ExternalInput")
    cdk = nc.dram_tensor("cdk", [128, 16 * 128], BF, kind="ExternalInput")
    cdv = nc.dram_tensor("cdv", [128, 16 * 128], BF, kind="ExternalInput")
    wo = nc.dram_tensor("wo", [DG, D], BF, kind="ExternalInput")  # g_rms folded
    lng = nc.dram_tensor("lng", [128, D], BF, kind="ExternalInput")
    lnb = nc.dram_tensor("lnb", [128, D], BF, kind="ExternalInput")
    ident_in = nc.dram_tensor("ident", [128, 128], BF, kind="ExternalInput")
    id2_in = nc.dram_tensor("id2f", [2, 2], F32, kind="ExternalInput")
    ones_in = nc.dram_tensor("ones", [128, 128], BF, kind="ExternalInput")
    mlo_in = nc.dram_tensor("mlo", [128, 128], BF, kind="ExternalInput")  # +(j<i)
    mup_in = nc.dram_tensor("mup", [128, 128], BF, kind="ExternalInput")  # +(j>i)
    mui_in = nc.dram_tensor("mui", [128, 128], BF, kind="ExternalInput")  # (j>=i)
    y_out = nc.dram_tensor("y_out", [TH, D], BF, kind="ExternalOutput")

    with tile.TileContext(nc) as tc, ExitStack() as top:
        top.enter_context(nc.allow_low_precision(
            reason="bf16 pipeline validated 8.3e-3 rel vs 2e-2 tol in sim"))
        const = top.enter_context(tc.tile_pool(name="const", bufs=1))
        psum = top.enter_context(tc.tile_pool(name="psum", bufs=2, space="PSUM"))
        dram = top.enter_context(tc.tile_pool(name="dram", bufs=1, space="DRAM"))

        def ps_pay():
            return psum.tile([128, 512], F32, tag="pay", name="pay")

        def ps_pg():
            return psum.tile([128, 256], F32, tag="pg", name="pg")

        def ps_med():
            return psum.tile([128, 512], F32, tag="pmed", name="pmed")

        def ps_small():
            return psum.tile([128, 512], F32, tag="psmall", name="psmall")

        # ---- stage-A weights + x first (startup critical path) ----
        wpool = top.enter_context(tc.tile_pool(name="wpool", bufs=1))
        xbp = top.enter_context(tc.tile_pool(name="xbp", bufs=2))
        WT = {}

        def load_w(nm, wsrc):
            wt = wpool.tile([128, 8 * DG], BF, tag=f"w{nm}", name=f"w{nm}")
            nc.sync.dma_start(wt[:].rearrange("p (k d) -> p k d", k=8),
                              wsrc[:].rearrange("(k p) d -> p k d", p=128))
            WT[nm] = wt

        def load_xb(t0):
            xb = xbp.tile([128, 8 * seg], BF, tag="xb", name="xb")
            nc.sync.dma_start(
                xb[:].rearrange("p (k t) -> p k t", k=8),
                xt[:].rearrange("(k p) t -> p k t", p=128)[:, :, t0:t0 + seg])
            return xb

        load_w("q", wq)
        xb0 = load_xb(0)
        WB2 = wpool.tile([128, 16], BF, tag="wb2", name="wb2")
        nc.sync.dma_start(WB2[:].rearrange("p (k j) -> p k j", k=8),
                          wb2[:].rearrange("(k p) j -> p k j", p=128))
        load_w("k", wk)
        load_w("v", wv)

        # ---- constants ----
        IB = const.tile([128, 128], BF, tag="ib", name="ib")
        nc.sync.dma_start(IB[:], ident_in[:])
        ONES = const.tile([128, 128], BF, tag="ones", name="ones")
        nc.sync.dma_start(ONES[:], ones_in[:])
        MLOP2 = const.tile([128, 256], BF, tag="mlop", name="mlop")
        nc.sync.dma_start(MLOP2[:, 0:128], mlo_in[:])
        nc.sync.dma_start(MLOP2[:, 128:256], mlo_in[:])
        MUPP2 = const.tile([128, 256], BF, tag="mupp", name="mupp")
        nc.sync.dma_start(MUPP2[:, 0:128], mup_in[:])
        nc.sync.dma_start(MUPP2[:, 128:256], mup_in[:])
        MUI2 = const.tile([128, 256], BF, tag="mui", name="mui")
        nc.sync.dma_start(MUI2[:, 0:128], mui_in[:])
        nc.sync.dma_start(MUI2[:, 128:256], mui_in[:])
        EPS1 = const.tile([128, 1], F32, tag="eps1", name="eps1")
        nc.gpsimd.memset(EPS1[:], 1e-6)
        EPSL = const.tile([128, 1], F32, tag="epsl", name="epsl")
        nc.gpsimd.memset(EPSL[:], EPS)
        ONE32 = const.tile([1, 1], F32, tag="one32", name="one32")
        nc.gpsimd.memset(ONE32[:], 1.0)
        ID2 = const.tile([2, 2], F32, tag="id2", name="id2")
        nc.sync.dma_start(ID2[:], id2_in[:])
        CD = {}
        for nm, cd in (("q", cdq), ("k", cdk), ("v", cdv)):
            CD[nm] = const.tile([128, 16 * 128], BF, tag=f"cd{nm}", name=f"cd{nm}")
            nc.sync.dma_start(CD[nm][:], cd[:])

        # beta rows + q/k/v stay in SBUF (bf16, d-major, col = j*T + t)
        BT = [const.tile([1, T], F32, tag=f"BT{h}", name=f"BT{h}") for h in range(2)]
        QKV = {}
        for nm in ("q", "k", "v"):
            QKV[nm] = const.tile([128, 4 * T], BF, tag=f"qkv{nm}", name=f"qkv{nm}")

        # ================= stage A: projections + conv + silu =================
        cvp = top.enter_context(tc.tile_pool(name="cvp", bufs=3))

        def conv_tail(st):
            """all 4 conv taps as diag-matmuls on PE; silu straight from PSUM."""
            nm, j, t0, u = st
            ps2 = ps_pg()
            for tap in range(4):
                nc.tensor.matmul(ps2[:, 0:TB],
                                 CD[nm][:, (4 * j + tap) * 128:
                                           (4 * j + tap + 1) * 128],
                                 u[:, 1 + tap:TB + 1 + tap],
                                 start=(tap == 0), stop=(tap == 3))
            nc.scalar.activation(QKV[nm][:, j * T + t0:j * T + t0 + TB],
                                 ps2[:, 0:TB], ACTF.Silu)

        pend_conv = []

        def emit_tb(tb):
            t0 = tb * TB
            xb = xb0 if tb == 0 else load_xb(t0)
            for h in range(2):
                psb = ps_small()
                for k in range(8):
                    nc.tensor.matmul(psb[0:1, 0:TB],
                                     WB2[:, k * 2 + h:k * 2 + h + 1],
                                     xb[:, k * seg + 4:(k + 1) * seg],
                                     start=(k == 0), stop=(k == 7))
                bth = cvp.tile([1, TB], F32, tag="bth", name="bth")
                nc.scalar.activation(bth[0:1, 0:TB], psb[0:1, 0:TB],
                                     ACTF.Tanh, scale=0.5)
                nc.vector.tensor_scalar(BT[h][0:1, t0:t0 + TB], bth[0:1, 0:TB],
                                        0.5, 0.5, ALU.mult, ALU.add)

            for nm in ("q", "k", "v"):
                for j in range(4):
                    ps = ps_pay()
                    for k in range(8):
                        nc.tensor.matmul(ps[:, 0:seg],
                                         WT[nm][:, k * DG + j * 128:
                                                    k * DG + (j + 1) * 128],
                                         xb[:, k * seg:(k + 1) * seg],
                                         start=(k == 0), stop=(k == 7))
                    # conv[t] = sum_i cw[i]*pre[t-3+i]; ps col (t-t0+4)
                    u = cvp.tile([128, seg], BF, tag="u", name="u")
                    nc.vector.tensor_copy(u[:, 1:seg], ps[:, 1:seg])
                    if pend_conv:
                        conv_tail(pend_conv.pop())
                    pend_conv.append((nm, j, t0, u))

        for _tb in range(n_tb):
            emit_tb(_tb)
        while pend_conv:
            conv_tail(pend_conv.pop())

        # ================= chunk stage: delta rule =================
        ckx = top.enter_context(ExitStack())
        work = ckx.enter_context(tc.tile_pool(name="work", bufs=3))
        spool = ckx.enter_context(tc.tile_pool(name="spool", bufs=2))
        ohp = ckx.enter_context(tc.tile_pool(name="ohp", bufs=2))
        wop = ckx.enter_context(tc.tile_pool(name="wop", bufs=1))
        xcp = ckx.enter_context(tc.tile_pool(name="xcp", bufs=2))

        WO = [wop.tile([128, D], BF, tag=f"wo{k}", name=f"wo{k}") for k in range(4)]
        for k in range(4):
            nc.sync.dma_start(WO[k][:], wo[k * 128:(k + 1) * 128, :])

        ydr = dram.tile([T, D], BF, tag="ydr", name="ydr")

        S = {}
        for h in range(2):
            S[h] = spool.tile([128, 2 * DH], BF, tag=f"S{h}", name=f"S{h}")
            nc.gpsimd.memset(S[h][:], 0.0)

        blocks = _rs_blocks(T)
        ybase = []
        off = 0
        for s, e in blocks:
            half = (e - s) * CK // 2
            ybase.append((off, half))
            off += half
        yhb = [dram.tile([ybase[rb][1], D], BF, tag=f"yhb{rb}", name=f"yhb{rb}")
               for rb in range(len(blocks))]

        LNG = const.tile([128, D], BF, tag="lng", name="lng")
        nc.sync.dma_start(LNG[:], lng[:])
        LNB = const.tile([128, D], BF, tag="lnb", name="lnb")
        nc.sync.dma_start(LNB[:], lnb[:])
        lnp = ckx.enter_context(tc.tile_pool(name="lnp", bufs=2))

        def dt_ap(nm, h, i, cc):
            """single d-tile [128, 128]"""
            j = 2 * h + i
            return QKV[nm][:, j * T + cc.start:j * T + cc.stop]

        def prep2(c):
            """Per-chunk prep for BOTH heads; raw-k Gram with scales folded
            into per-partition columns (CB2) and row-broadcast masks."""
            cc = slice(c * CK, (c + 1) * CK)

            # -- l2 norm sums: SQ2 [q0|k0|q1|k1|q2|k2|q3|k3] (j = 2h+i) --
            SQ2 = work.tile([128, 1024], BF, tag="SQ", name="SQ", bufs=2)
            sqw = SQ2[:].rearrange("p (j t) -> p j t", j=4)
            qa = QKV["q"][:].rearrange("p (j t) -> p j t", j=4)[:, :, cc]
            ka = QKV["k"][:].rearrange("p (j t) -> p j t", j=4)[:, :, cc]
            nc.vector.tensor_tensor(sqw[:, :, 0:128], qa, qa, ALU.mult)
            nc.vector.tensor_tensor(sqw[:, :, 128:256], ka, ka, ALU.mult)
            psn = ps_small()  # [1,512] = [nq0|nk0|nq1|nk1]
            for h in range(2):
                for i in range(2):
                    nc.tensor.matmul(psn[0:1, h * 256:(h + 1) * 256],
                                     ONES[:, 0:1],
                                     SQ2[:, (2 * h + i) * 256:
                                            (2 * h + i + 1) * 256],
                                     start=(i == 0), stop=(i == 1))
            sqr = work.tile([1, 512], F32, tag="sqr", name="sqr")
            nc.scalar.activation(sqr[:], psn[0:1, 0:512], ACTF.Sqrt,
                                 bias=EPS1[0:1, 0:1])
            R3f = work.tile([1, 512], F32, tag="R3f", name="R3f")
            nc.vector.reciprocal_approx_fast(R3f[:], sqr[:])
            # RCAT [1,512] = [rk0|mb0|rk1|mb1]  (mb = -beta*rk)
            RCAT = work.tile([1, 512], BF, tag="RCAT", name="RCAT")
            rcw = RCAT[0:1].rearrange("p (i t) -> p i t", i=2)
            r3w = R3f[0:1].rearrange("p (i t) -> p i t", i=2)
            nc.vector.tensor_copy(rcw[:, :, 0:128], r3w[:, :, 128:256])
            for h in range(2):
                nc.vector.scalar_tensor_tensor(
                    RCAT[0:1, h * 256 + 128:(h + 1) * 256],
                    R3f[0:1, h * 256 + 128:(h + 1) * 256],
                    -1.0, BT[h][0:1, cc], ALU.mult, ALU.mult)
            psbr = ps_small()
            nc.tensor.matmul(psbr[0:128, 0:512], ONES[0:1, :], RCAT[:],
                             start=True, stop=True)
            RB = work.tile([128, 512], BF, tag="RB", name="RB", bufs=2)
            nc.scalar.activation(RB[:], psbr[0:128, 0:512], ACTF.Copy)
            # columns: CB2 [128,6]: per h: rq(3h) b(3h+1) rk(3h+2); MBC: mb
            pst1 = ps_small()
            for h in range(2):
                nc.tensor.transpose(pst1[0:128, 3 * h:3 * h + 1],
                                    R3f[0:1, h * 256:h * 256 + 128],
                                    ONE32[0:1, 0:1])
                nc.tensor.transpose(pst1[0:128, 3 * h + 1:3 * h + 2],
                                    BT[h][0:1, cc], ONE32[0:1, 0:1])
                nc.tensor.transpose(pst1[0:128, 3 * h + 2:3 * h + 3],
                                    R3f[0:1, h * 256 + 128:(h + 1) * 256],
                                    ONE32[0:1, 0:1])
            pstb = pst1[:, 6:8].bitcast(BF)  # [128, 4] bf16 view
            for h in range(2):
                nc.tensor.transpose(pstb[0:128, 2 * h:2 * h + 1],
                                    RCAT[0:1, h * 256 + 128:(h + 1) * 256],
                                    ONES[0:1, 0:1])
            CB2 = work.tile([128, 6], F32, tag="CB", name="CB", bufs=6)
            nc.scalar.copy(CB2[:], pst1[0:128, 0:6])
            MBC = work.tile([128, 2], F32, tag="MBC", name="MBC", bufs=6)
            nc.vector.tensor_copy(
                MBC[:].rearrange("p (h x) -> p h x", x=1),
                pstb[0:128].rearrange("p (h x) -> p h x", h=2)[:, :, 0:1])
            cbh = CB2[:].rearrange("p (h x) -> p h x", h=2)
            r2dh2 = work.tile([128, 2], F32, tag="r2dh", name="r2dh", bufs=6)
            nc.vector.scalar_tensor_tensor(
                r2dh2[:].rearrange("p (h x) -> p h x", x=1),
                cbh[:, :, 0:1], 1.0 / DH, cbh[:, :, 0:1], ALU.mult, ALU.mult)

            # -- row-broadcast masks: MLORK = mlo*rk_rows, MUPBK = mup*mb_rows
            rb2 = RB[:].rearrange("p (i t) -> p i t", i=2)
            MLORK = work.tile([128, 256], BF, tag="MLORK", name="MLORK", bufs=2)
            nc.vector.tensor_tensor(
                MLORK[:].rearrange("p (i t) -> p i t", i=2),
                MLOP2[:].rearrange("p (i t) -> p i t", i=2),
                rb2[:, :, 0:128], ALU.mult)
            MUPBK = work.tile([128, 256], BF, tag="MUPBK", name="MUPBK", bufs=2)
            nc.vector.tensor_tensor(
                MUPBK[:].rearrange("p (i t) -> p i t", i=2),
                MUPP2[:].rearrange("p (i t) -> p i t", i=2),
                rb2[:, :, 128:256], ALU.mult)

            # -- Gram (symmetric, raw k): psg [G_h0 | G_h1] --
            psg = ps_pg()
            for h in range(2):
                for i in range(2):
                    nc.tensor.matmul(psg[:, h * 128:(h + 1) * 128],
                                     dt_ap("k", h, i, cc),
                                     dt_ap("k", h, i, cc),
                                     start=(i == 0), stop=(i == 1))
            An2 = work.tile([128, 256], BF, tag="An", name="An", bufs=2)
            ATn2 = work.tile([128, 256], BF, tag="ATn", name="ATn", bufs=2)
            for h in range(2):
                sl = slice(h * 128, (h + 1) * 128)
                nc.vector.scalar_tensor_tensor(An2[:, sl], psg[:, sl],
                                               MBC[:, h:h + 1],
                                               MLORK[:, sl], ALU.mult, ALU.mult)
                nc.vector.scalar_tensor_tensor(ATn2[:, sl], psg[:, sl],
                                               CB2[:, 3 * h + 2:3 * h + 3],
                                               MUPBK[:, sl], ALU.mult, ALU.mult)

            # -- truncated inverse transpose: TpT = (I+A4T)(I+A2T)(I-AT) --
            pp = ps_pay()  # [P2_0|P2T_0|P2_1|P2T_1]
            for h in range(2):
                sl = slice(h * 128, (h + 1) * 128)
                nc.tensor.matmul(pp[:, h * 256:h * 256 + 128], ATn2[:, sl],
                                 An2[:, sl], start=True, stop=True)
                nc.tensor.matmul(pp[:, h * 256 + 128:(h + 1) * 256],
                                 An2[:, sl], ATn2[:, sl], start=True, stop=True)
            PP = work.tile([128, 512], BF, tag="PP", name="PP", bufs=2)
            nc.scalar.copy(PP[:], pp[:, 0:512])
            pr1 = ps_pg()
            for h in range(2):
                sl = slice(h * 128, (h + 1) * 128)
                nc.tensor.matmul(pr1[:, sl], PP[:, h * 256:h * 256 + 128],
                                 PP[:, h * 256 + 128:(h + 1) * 256],
                                 start=True, stop=False)
                nc.tensor.matmul(pr1[:, sl], IB[:], IB[:], start=False,
                                 stop=True)
            R1 = work.tile([128, 256], BF, tag="R1", name="R1", bufs=2)
            nc.scalar.copy(R1[:], pr1[:, 0:256])
            pr2 = ps_pg()
            for h in range(2):
                sl = slice(h * 128, (h + 1) * 128)
                nc.tensor.matmul(pr2[:, sl], PP[:, h * 256:h * 256 + 128],
                                 R1[:, sl], start=True, stop=False)
                nc.tensor.matmul(pr2[:, sl], IB[:], R1[:, sl], start=False,
                                 stop=True)
            R2 = work.tile([128, 256], BF, tag="R2", name="R2", bufs=2)
            nc.scalar.copy(R2[:], pr2[:, 0:256])
            pr3 = ps_pg()
            for h in range(2):
                sl = slice(h * 128, (h + 1) * 128)
                nc.tensor.matmul(pr3[:, sl], An2[:, sl], R2[:, sl],
                                 start=True, stop=False)
                nc.tensor.matmul(pr3[:, sl], IB[:], R2[:, sl], start=False,
                                 stop=True)
            TTt2 = work.tile([128, 256], BF, tag="TTt", name="TTt", bufs=5)
            nc.scalar.copy(TTt2[:], pr3[:, 0:256])

            # -- MT' = triu(k_raw^T q_raw) (rk rides on U') --
            psmt = ps_pg()
            for h in range(2):
                for i in range(2):
                    nc.tensor.matmul(psmt[:, h * 128:(h + 1) * 128],
                                     dt_ap("k", h, i, cc),
                                     dt_ap("q", h, i, cc),
                                     start=(i == 0), stop=(i == 1))
            MT2 = work.tile([128, 256], BF, tag="MT", name="MT", bufs=5)
            nc.vector.tensor_tensor(MT2[:], psmt[:, 0:256], MUI2[:], ALU.mult)

            # -- beta*V (t-major) via PE transpose --
            vw = ps_med()
            vwb = vw[:, 0:256].bitcast(BF)  # [128, 512] bf16 view
            for h in range(2):
                for i in range(2):
                    nc.tensor.transpose(vwb[:, (2 * h + i) * 128:
                                               (2 * h + i + 1) * 128],
                                        dt_ap("v", h, i, cc), IB[:])
            Vtb2 = work.tile([128, 512], BF, tag="Vtb", name="Vtb", bufs=5)
            for h in range(2):
                nc.scalar.activation(Vtb2[:, h * 256:(h + 1) * 256],
                                     vwb[:, h * 256:(h + 1) * 256],
                                     ACTF.Copy, scale=CB2[:, 3 * h + 1:3 * h + 2])

            # -- raw K t-major (rk rides on U'); last chunk never updates S
            Kh2 = None
            if c + 1 < n_ck:
                pskt = ps_med()
                psktb = pskt[:, 0:256].bitcast(BF)
                for h in range(2):
                    for i in range(2):
                        nc.tensor.transpose(psktb[:, (2 * h + i) * 128:
                                                    (2 * h + i + 1) * 128],
                                            dt_ap("k", h, i, cc), IB[:])
                Kh2 = work.tile([128, 512], BF, tag="Kh", name="Kh", bufs=5)
                nc.vector.tensor_copy(Kh2[:], psktb[:])

            return dict(TTt=TTt2, Vtb=Vtb2, MT=MT2, Kh=Kh2, CB=CB2,
                        MBC=MBC, r2dh=r2dh2)

        def spart2(c, Pd):
            cc = slice(c * CK, (c + 1) * CK)
            TTt2, Vtb2, MT2 = Pd["TTt"], Pd["Vtb"], Pd["MT"]
            Kh2, CB2, r2dh2 = Pd["Kh"], Pd["CB"], Pd["r2dh"]
            MBC = Pd["MBC"]

            # kS = k_raw^T S; W2b = (kS * mb) + beta*Vt  (mb = -beta*rk)
            psw = ps_med()
            for h in range(2):
                for i in range(2):
                    nc.tensor.matmul(psw[:, h * 256:(h + 1) * 256],
                                     dt_ap("k", h, i, cc),
                                     S[h][:, i * 256:(i + 1) * 256],
                                     start=(i == 0), stop=(i == 1))
            W2b2 = work.tile([128, 512], BF, tag="W2b", name="W2b", bufs=2)
            for h in range(2):
                sl = slice(h * 256, (h + 1) * 256)
                nc.vector.scalar_tensor_tensor(W2b2[:, sl], psw[:, sl],
                                               MBC[:, h:h + 1],
                                               Vtb2[:, sl], ALU.mult, ALU.add)
            # U' = rk * (Tp' W2b)
            pu = ps_med()
            for h in range(2):
                sl = slice(h * 256, (h + 1) * 256)
                nc.tensor.matmul(pu[:, sl], TTt2[:, h * 128:(h + 1) * 128],
                                 W2b2[:, sl], start=True, stop=True)
            U2 = work.tile([128, 512], BF, tag="U", name="U", bufs=2)
            for h in range(2):
                sl = slice(h * 256, (h + 1) * 256)
                nc.scalar.activation(U2[:, sl], pu[:, sl], ACTF.Copy,
                                     scale=CB2[:, 3 * h + 2:3 * h + 3])

            # O_raw (t-major) = q_raw S + MT' U'; per-head RMS, r_q folded
            pso = ps_med()
            for h in range(2):
                sl = slice(h * 256, (h + 1) * 256)
                for i in range(2):
                    nc.tensor.matmul(pso[:, sl], dt_ap("q", h, i, cc),
                                     S[h][:, i * 256:(i + 1) * 256],
                                     start=(i == 0), stop=False)
                nc.tensor.matmul(pso[:, sl], MT2[:, h * 128:(h + 1) * 128],
                                 U2[:, sl], start=False, stop=True)
            waste = work.tile([128, 512], BF, tag="waste", name="waste", bufs=1)
            sso2 = work.tile([128, 2], F32, tag="sso", name="sso")
            for h in range(2):
                nc.scalar.activation(waste[:, h * 256:(h + 1) * 256],
                                     pso[:, h * 256:(h + 1) * 256],
                                     ACTF.Square, accum_out=sso2[:, h:h + 1])
            ssp = work.tile([128, 2], F32, tag="ssp", name="ssp")
            nc.vector.tensor_tensor(ssp[:], sso2[:], r2dh2[:], ALU.mult)
            sdo = work.tile([128, 2], F32, tag="sdo", name="sdo")
            nc.scalar.activation(sdo[:], ssp[:], ACTF.Sqrt, bias=EPSL[:])
            rcoi = work.tile([128, 2], F32, tag="rcoi", name="rcoi")
            nc.vector.reciprocal_approx_fast(rcoi[:], sdo[:])
            rco = work.tile([128, 2], F32, tag="rco", name="rco")
            cbh = CB2[:].rearrange("p (h x) -> p h x", h=2)
            nc.vector.tensor_tensor(
                rco[:].rearrange("p (h x) -> p h x", x=1),
                rcoi[:].rearrange("p (h x) -> p h x", x=1),
                cbh[:, :, 0:1], ALU.mult)
            Ohn2 = work.tile([128, 512], BF, tag="Ohn", name="Ohn", bufs=2)
            for h in range(2):
                sl = slice(h * 256, (h + 1) * 256)
                nc.vector.tensor_scalar(Ohn2[:, sl], pso[:, sl],
                                        rco[:, h:h + 1], None, ALU.mult)
            psot = ps_pg()
            psob = psot[:, 0:256].bitcast(BF)  # [128, 512] bf16 view
            for jj in range(4):
                nc.tensor.transpose(psob[:, jj * 128:(jj + 1) * 128],
                                    Ohn2[:, jj * 128:(jj + 1) * 128], IB[:])
            OhT2 = ohp.tile([128, 512], BF, tag="OhT", name="OhT")
            nc.scalar.copy(OhT2[:], psob[:])

            # S += k_raw^T U'  (skip on the final chunk)
            if c + 1 < n_ck:
                for h in range(2):
                    ktds = ps_med() if h == 0 else ps_pay()
                    for i in range(2):
                        reg = slice(i * 256, (i + 1) * 256)
                        nc.tensor.matmul(ktds[:, reg],
                                         Kh2[:, (2 * h + i) * 128:
                                                (2 * h + i + 1) * 128],
                                         U2[:, h * 256:(h + 1) * 256],
                                         start=True, stop=True)
                    Snew = spool.tile([128, 2 * DH], BF, tag=f"S{h}",
                                      name=f"S{h}")
                    nc.vector.tensor_tensor(Snew[:], S[h][:], ktds[:, 0:512],
                                            ALU.add)
                    S[h] = Snew
            return OhT2

        def ln_rows(src_ap, dst_rows, nrows):
            yr = lnp.tile([128, D], BF, tag="yr", name="yr")
            nc.gpsimd.dma_start(yr[0:nrows, :], src_ap)
            srow = lnp.tile([128, 1], F32, tag="srow", name="srow")
            nc.vector.tensor_reduce(srow[0:nrows, :], yr[0:nrows, :],
                                    mybir.AxisListType.X, ALU.add)
            ysq = lnp.tile([128, D], BF, tag="ysq", name="ysq")
            ssq = lnp.tile([128, 1], F32, tag="ssq", name="ssq")
            nc.scalar.activation(ysq[0:nrows, :], yr[0:nrows, :],
                                 ACTF.Square, accum_out=ssq[0:nrows, :])
            mneg = lnp.tile([128, 1], F32, tag="mneg", name="mneg")
            nc.scalar.mul(mneg[0:nrows, :], srow[0:nrows, :], -1.0 / D)
            mu2 = lnp.tile([128, 1], F32, tag="mu2", name="mu2")
            nc.vector.tensor_tensor(mu2[0:nrows, :], mneg[0:nrows, :],
                                    mneg[0:nrows, :], ALU.mult)
            var = lnp.tile([128, 1], F32, tag="var", name="var")
            nc.vector.scalar_tensor_tensor(var[0:nrows, :], ssq[0:nrows, :],
                                           1.0 / D, mu2[0:nrows, :], ALU.mult,
                                           ALU.subtract)
            sdv = lnp.tile([128, 1], F32, tag="sdv", name="sdv")
            nc.scalar.activation(sdv[0:nrows, :], var[0:nrows, :], ACTF.Sqrt,
                                 bias=EPSL[0:nrows, :])
            rstd = lnp.tile([128, 1], F32, tag="rstd", name="rstd")
            nc.vector.reciprocal_approx_fast(rstd[0:nrows, :], sdv[0:nrows, :])
            bcl = lnp.tile([128, 1], F32, tag="bcl", name="bcl")
            nc.vector.tensor_tensor(bcl[0:nrows, :], mneg[0:nrows, :],
                                    rstd[0:nrows, :], ALU.mult)
            yn = lnp.tile([128, D], BF, tag="yn", name="yn")
            nc.scalar.activation(yn[0:nrows, :], yr[0:nrows, :], ACTF.Identity,
                                 scale=rstd[0:nrows, :], bias=bcl[0:nrows, :])
            yg = lnp.tile([128, D], BF, tag="ysq", name="yg")
            nc.vector.tensor_tensor(yg[0:nrows, :], yn[0:nrows, :],
                                    LNG[0:nrows, :], ALU.mult)
            yfin = lnp.tile([128, D], BF, tag="yr", name="yfin")
            nc.vector.tensor_tensor(yfin[0:nrows, :], yg[0:nrows, :],
                                    LNB[0:nrows, :], ALU.add)
            nc.gpsimd.dma_start(y_out[dst_rows, :], yfin[0:nrows, :])

        def emit_ln(rb):
            yoff, half = ybase[rb]
            for r0 in range(0, half, 128):
                nr = min(128, half - r0)
                ln_rows(yhb[rb][r0:r0 + nr, :],
                        slice(yoff + r0, yoff + r0 + nr), nr)

        pending = []
        Pmap = {}
        for _c in range(min(3, n_ck)):
            Pmap[_c] = prep2(_c)
        for c in range(n_ck):
            cc = slice(c * CK, (c + 1) * CK)
            for rb, done_c in list(pending):
                defer = 3 if blocks[rb][1] <= 12 else 2
                if done_c <= c - defer:
                    emit_ln(rb)
                    pending.remove((rb, done_c))
            if c + 3 < n_ck:
                Pmap[c + 3] = prep2(c + 3)
            oht = spart2(c, Pmap.pop(c))
            # -- partial y = o @ Wo + 0.5x for this chunk --
            xc = xcp.tile([128, D], BF, tag="xc", name="xc")
            nc.sync.dma_start(xc[:], xtm[cc, :])
            for n in range(2):
                psy = ps_pay()
                for kk in range(4):
                    nc.tensor.matmul(psy[:], oht[:, kk * 128:(kk + 1) * 128],
                                     WO[kk][:, n * 512:(n + 1) * 512],
                                     start=(kk == 0), stop=(kk == 3))
                ysb = work.tile([128, 512], BF, tag="ysb", name="ysb")
                if n == 0:
                    nc.vector.tensor_tensor(ysb[:], psy[:],
                                            xc[:, n * 512:(n + 1) * 512],
                                            ALU.add)
                else:
                    nc.scalar.activation(ysb[:], psy[:], ACTF.Identity,
                                         bias=None, scale=1.0,
                                         accum_out=None) if False else                         nc.vector.tensor_tensor(ysb[:], psy[:],
                                                xc[:, n * 512:(n + 1) * 512],
                                                ALU.add)
                nc.sync.dma_start(ydr[c * CK:(c + 1) * CK, n * 512:(n + 1) * 512],
                                  ysb[:])
            # -- overlapped ReduceScatter; LayerNorm deferred 2 chunks --
            for rb, (s, e) in enumerate(blocks):
                if c + 1 == e:
                    nc.gpsimd.collective_compute(
                        "ReduceScatter", ALU.add,
                        replica_groups=[[0, 1], [2, 3], [4, 5], [6, 7]],
                        ins=[ydr[s * CK:e * CK, :]], outs=[yhb[rb].opt()],
                    )
                    pending.append((rb, c))
        for rb, done_c in pending:
            emit_ln(rb)

    nc.compile()
    return nc


def _shard(inputs, T=T_FULL):
    import ml_dtypes
    BFNP = ml_dtypes.bfloat16
    x = np.asarray(inputs["x"], dtype=np.float32)
    bf = lambda a: np.ascontiguousarray(np.asarray(a, dtype=np.float32)
                                        .astype(BFNP))
    f32 = lambda a: np.ascontiguousarray(np.asarray(a), dtype=np.float32)
    Wq, Wk, Wv = inputs["Wq"], inputs["Wk"], inputs["Wv"]
    Wb, Wo = inputs["Wb"], inputs["Wo"]
    cq, ck, cv = inputs["conv_q"], inputs["conv_k"], inputs["conv_v"]
    g_rms, ln_g, ln_b = (np.asarray(inputs["g_rms"], np.float32),
                         np.asarray(inputs["ln_g"], np.float32),
                         np.asarray(inputs["ln_b"], np.float32))

    ident = np.eye(128, dtype=np.float32)
    ii, jj = np.indices((128, 128))
    mlo = (jj < ii).astype(np.float32)
    mup = (jj > ii).astype(np.float32)
    mui = (jj >= ii).astype(np.float32)
    grms_col = np.tile(g_rms, 2)[:, None]  # [DG, 1] scales for Wo rows

    def conv_diag(cw, gs):
        """[128, 16*128]: per j-tile, diag(c0_j)..diag(c3_j)."""
        cg = np.asarray(cw, np.float32)[:, gs]  # [K, DG]
        blocks = []
        for j in range(4):
            for tap in range(4):
                blocks.append(np.diag(cg[tap, j * 128:(j + 1) * 128]))
        return np.concatenate(blocks, axis=1)

    in_maps = []
    for c in range(N_CORES):
        b, g = c // 2, c % 2
        gs = slice(g * DG, (g + 1) * DG)
        in_maps.append({
            "xt": bf(np.concatenate([np.zeros((D, 4), np.float32),
                                     x[b, :T].T], axis=1)),
            "xtm": bf(0.5 * x[b, :T]),
            "wq": bf(np.asarray(Wq)[:, gs]), "wk": bf(np.asarray(Wk)[:, gs]),
            "wv": bf(np.asarray(Wv)[:, gs]),
            "wb2": bf(np.asarray(Wb)[:, 2 * g:2 * g + 2]),
            "cqT": f32(np.asarray(cq)[:, gs].T),
            "ckT": f32(np.asarray(ck)[:, gs].T),
            "cvT": f32(np.asarray(cv)[:, gs].T),
            "cdq": bf(conv_diag(cq, gs)),
            "cdk": bf(conv_diag(ck, gs)),
            "cdv": bf(conv_diag(cv, gs)),
            "wo": bf(np.asarray(Wo)[gs, :] * grms_col),
            "lng": bf(np.tile(ln_g[None, :], (128, 1))),
            "lnb": bf(np.tile(ln_b[None, :], (128, 1))),
            "ident": bf(ident), "ones": bf(np.ones((128, 128), np.float32)),
            "id2f": f32(np.eye(2, dtype=np.float32)),
            "mlo": bf(mlo), "mup": bf(mup), "mui": bf(mui),
        })
    return in_maps


def kernel(**inputs):
    from concourse.bass_utils import run_bass_kernel_spmd
    T = T_FULL
    if "nc" not in _cache:
        _cache["nc"] = _build(T)
    nc = _cache["nc"]
    in_maps = _shard(inputs, T)
    res = run_bass_kernel_spmd(nc, in_maps, core_ids=list(range(N_CORES)))
    out = np.empty((B, T, D), dtype=np.float32)
    ridx = _rows_idx(T)
    for c in range(N_CORES):
        b, g = c // 2, c % 2
        out[b, ridx[g]] = np.asarray(res.results[c]["y_out"], dtype=np.float32)
    return out

